# revision 35
# baseline (speedup 1.0000x reference)
"""BERT_BiLSTM_CRF loss (CRF NLL) Trainium2 kernel — TensorE-forward version.

Self-contained: kernel(**inputs) takes FULL inputs, shards batch across 8
NeuronCores (128 seqs/core), returns the scalar mean loss.

Forward: the CRF recurrence S_t = diag(ef_t) E S_{t-1} (exp domain,
G-centered emissions) runs on the TensorEngine in a transposed layout:
partitions = 16 groups x 8 states (7 real + 1 dummy), columns = sets x 128
seqs; blk = set*16+g gives NBLK=128 time blocks of C=16. Per step and
column-half: one block-diagonal [128x128]x[128,512] matmul + one DVE mult
by transposed emissions EFT (built by DMA-XBAR transposes of exp(feats-G)).
The stationary's dummy columns hold E8 = exp(trans[STOP,:]) and the dummy
emission is 1.0, so each matmul's dummy output rows carry the previous
step's final-score dot d = E8 . S — written per step to DRAM (16 rows via
partition-strided DMA). Probe pass (last 4 taus, uniform start) seeds all
blocks (unnormalized); per-block delta = ln(end mass) - ln(probe mass)
telescopes into the per-seq offset via triangular 16x16 matmul prefix.
fwd = ln(d*) + Lam* + G*len, d*/Lam* fetched by indirect DMA at len-1.
Validated vs the exact reference in numpy (bf16 chain: loss rel 8e-7).

Gold scores: sum_t feat[t,tag_t] via eq-mask/mult/reduce on GPSIMD;
sum_t trans[tag_t,tag_{t-1}] via exact degree-6 Horner polynomials
(host-solved Vandermonde coefficients; two degree-3 custom DVE ops per row,
quadratic coeffs inlined as immediates) + eq-masked accumulation per row.
"""

import numpy as np

B, T, K = 1024, 2048, 9
NCORES = 8
BL = B // NCORES          # sequences per core (=128 partitions)
KA = 7                    # active states
JP = 8                    # padded state dim (j=7 is the d-carrier dummy)
START, STOP = 7, 8
G = 2.4                   # per-step log growth centering
C = 16                    # block length
NBLK = T // C             # 128 blocks
NSET = NBLK // 16         # 8 sets; blk = set*16 + g
P = 128                   # partitions in transposed layout: p = g*8 + j
COLS = NSET * BL          # 1024 columns: n = set*128 + b
HC = COLS // 2            # column half (4 sets)
FCH = T // 16             # feats DMA chunk (128 timesteps = 8 blocks)

_CACHE = {}
TRACE = False


def _register_dve_ops():
    import concourse.dve_ops as DO
    from concourse.dve_spec import Spec, Src0, Src1, C0, C1, C2, C3, eq, \
        lower, _spill_c3_to_src1
    from concourse.dve_uop import DveOpSpec
    from concourse.dve_spec import AluOp as SAluOp

    existing = {o.name: o for o in DO.OPS}

    def mk(name, spec, subdim=False):
        if name in existing:
            return existing[name]
        op = DO.DveOp(name, spec, subdim, uops_sha={})
        DO.OPS.append(op)
        DO.CUSTOM_DVE_SPECS[name] = spec
        DO._SUB_OPCODE_FOR_NAME[name] = DO._CUSTOM_DVE_ROW_BASE + len(DO.OPS) - 1
        for ver in ("v3", "v4"):
            r = DveOpSpec(name=name, opcode=DO.get_dve_sub_opcode(name),
                          uops=lower(spec, ver=ver), rd1_en=DO.has_src1(spec))
            op.uops_sha[ver] = r.sha(ver)
        return op

    def _eqmul_ref(in0, in1, s0, s1, imm2):
        out = (np.asarray(in0, np.float32) == s0) * np.asarray(in1, np.float32)
        acc = (s1 if isinstance(s1, float) else np.asarray(s1, np.float32)) \
            + out.sum(axis=1, keepdims=True)
        return out, acc

    def _h3top_ref(in0, in1, s0, s1, imm2):
        x = np.asarray(in0, np.float32)
        c3 = np.asarray(in1, np.float32)
        return s0 + x * (s1 + x * (imm2 + x * c3))

    def _h3step_ref(in0, in1, s0, s1, imm2):
        x = np.asarray(in0, np.float32)
        h = np.asarray(in1, np.float32)
        return s0 + x * (s1 + x * (imm2 + x * h))

    ops = {}
    # accum_out = s1 + sum_n (Src0==s0)*Src1
    ops["ANT_EQMUL_ACC"] = mk(
        "ANT_EQMUL_ACC",
        Spec(body=eq(Src0, C0) * Src1, accum=SAluOp.ADD, accum_init=C1,
             reference=_eqmul_ref))
    # out = s0 + x*(s1 + x*(imm2 + x*c3)) with c3 spilled to in1 (elem 0)
    ops["ANT_H3_TOP"] = mk(
        "ANT_H3_TOP",
        Spec(body=_spill_c3_to_src1(C0 + Src0 * (C1 + Src0 * (C2 + Src0 * C3))),
             reference=_h3top_ref))
    # out = s0 + x*(s1 + x*(imm2 + x*Src1))
    ops["ANT_H3_STEP"] = mk(
        "ANT_H3_STEP",
        Spec(body=C0 + Src0 * (C1 + Src0 * (C2 + Src0 * Src1)),
             reference=_h3step_ref))
    return ops


def _build_bass(coefs_host):
    import concourse.bass as bass
    import concourse.bacc as bacc
    import concourse.tile as tile
    import concourse.mybir as mybir

    OPS = _register_dve_ops()

    f32 = mybir.dt.float32
    bf16 = mybir.dt.bfloat16
    i32 = mybir.dt.int32
    AX = mybir.AxisListType
    OP = mybir.AluOpType
    AF = mybir.ActivationFunctionType

    nc = bacc.Bacc()

    feats = nc.dram_tensor("feats", [BL, T, K], f32, kind="ExternalInput")
    tagf = nc.dram_tensor("tagf", [BL, T], f32, kind="ExternalInput")
    lenf = nc.dram_tensor("lenf", [BL, 1], f32, kind="ExternalInput")
    leni = nc.dram_tensor("leni", [BL, 1], i32, kind="ExternalInput")
    trans = nc.dram_tensor("trans", [K, K], f32, kind="ExternalInput")
    # Horner coefficients (host Vandermonde): rows 0..6 = trans[j, x-1],
    # row 7 = trans[STOP, x-1], row 8 = trans[x-1, START]; all in x=tag+1.
    coefs = nc.dram_tensor("coefs", [9, 7], f32, kind="ExternalInput")
    outv = nc.dram_tensor("outv", [BL, 1], f32, kind="ExternalOutput")

    hist_d = nc.dram_tensor("hist_d", [(C + 1) * 16 * COLS, 1], bf16)
    offs_d = nc.dram_tensor("offs_d", [16 * COLS, 1], f32)

    iota_t_np = np.arange(T, dtype=np.float32).reshape(1, T)
    c_iota_t = nc.inline_tensor(iota_t_np, "c_iota_t")
    c_b32 = nc.inline_tensor(np.arange(BL, dtype=np.int32).reshape(BL, 1),
                             "c_b32")
    c_sTm1 = nc.inline_tensor((np.arange(BL, dtype=np.int64) * T - 1)
                              .astype(np.int32).reshape(BL, 1), "c_sTm1")
    onebd_np = np.zeros((P, 16), np.float32)
    for g in range(16):
        onebd_np[8 * g:8 * g + 7, g] = 1.0   # exclude dummy row j=7
    c_onebd = nc.inline_tensor(onebd_np, "c_onebd")
    tri16_np = np.triu(np.ones((16, 16), np.float32), 1)  # [k,m]=1 iff k<m
    c_tri16 = nc.inline_tensor(tri16_np, "c_tri16")
    c_ones16 = nc.inline_tensor(np.ones((16, 16), np.float32), "c_ones16")

    ch = coefs_host  # [9, 7] float, for inline immediates (quadratic coefs)

    with tile.TileContext(nc) as tc:
        import contextlib
        ctx = contextlib.ExitStack()
        with ctx:
            sing = ctx.enter_context(tc.tile_pool(name="sing", bufs=1))
            epool = ctx.enter_context(tc.tile_pool(name="epool", bufs=2))
            cpool = ctx.enter_context(tc.tile_pool(name="cpool", bufs=3))
            gpool = ctx.enter_context(tc.tile_pool(name="gpool", bufs=1))
            gq = ctx.enter_context(tc.tile_pool(name="gq", bufs=1))
            spool = ctx.enter_context(tc.tile_pool(name="spool", bufs=4))
            mmps = ctx.enter_context(
                tc.tile_pool(name="mmps", bufs=2, space="PSUM"))
            upps = ctx.enter_context(
                tc.tile_pool(name="upps", bufs=1, space="PSUM"))

            # ---------- tiny constants ----------
            negG = sing.tile([BL, 1], f32)
            nc.gpsimd.memset(negG[:], -G)
            coefb = sing.tile([BL, 63], f32)
            nc.gpsimd.dma_start(coefb[:], bass.AP(coefs, 0, [[0, BL], [1, 63]]))
            iota_t = sing.tile([BL, T], f32)
            nc.gpsimd.dma_start(iota_t[:], bass.AP(c_iota_t, 0, [[0, BL], [1, T]]))
            b32 = sing.tile([BL, 1], i32)
            nc.gpsimd.dma_start(b32[:], c_b32[:, :])
            sTm1 = sing.tile([BL, 1], i32)
            nc.gpsimd.dma_start(sTm1[:], c_sTm1[:, :])
            lenf_sb = sing.tile([BL, 1], f32)
            nc.gpsimd.dma_start(lenf_sb[:], lenf[:, :])
            leni_sb = sing.tile([BL, 1], i32)
            nc.gpsimd.dma_start(leni_sb[:], leni[:, :])

            # tags: needed early for gold
            tagf_sb = sing.tile([BL, T], f32)
            nc.sync.dma_start(tagf_sb[:], tagf[:, :])

            # E^T extended tile: EtA[i, j<7] = exp(trans[j, i]),
            # EtA[i, 7] = exp(trans[STOP, i])  (final-score column)
            t7x = sing.tile([7, 8], f32)
            nc.gpsimd.dma_start(t7x[:, 0:7], bass.AP(trans, 0, [[1, 7], [9, 7]]))
            nc.gpsimd.dma_start(t7x[:, 7:8],
                                bass.AP(trans, STOP * K, [[1, 7], [1, 1]]))
            EtA = sing.tile([7, 8], bf16)
            nc.scalar.activation(EtA[:], t7x[:], AF.Exp)
            # E7[j] = exp(trans[j, START]) on partitions j
            t7b = sing.tile([7, 1], f32)
            nc.gpsimd.dma_start(t7b[:], bass.AP(trans, START, [[9, 7], [1, 1]]))
            E7e = sing.tile([7, 1], f32)
            nc.scalar.activation(E7e[:], t7b[:], AF.Exp)
            # rsE[j] = sum_i E[j,i] (no matmul: row-major E tile + reduce)
            t7r = sing.tile([7, 7], f32)
            nc.gpsimd.dma_start(t7r[:], bass.AP(trans, 0, [[9, 7], [1, 7]]))
            Ete = sing.tile([7, 7], f32)
            nc.scalar.activation(Ete[:], t7r[:], AF.Exp)
            rs7 = sing.tile([7, 1], f32)
            nc.vector.tensor_reduce(out=rs7[:], in_=Ete[:], axis=AX.X, op=OP.add)
            rsE = sing.tile([P, 1], f32)
            nc.gpsimd.memset(rsE[:], 0.0)
            for g in range(16):
                nc.gpsimd.dma_start(rsE[8 * g:8 * g + 7, :], rs7[:, :])

            # ---------- stationary matrices ----------
            # EB blockdiag: col (g,j<7) = E^T block; col (g,7) = E8 (so each
            # matmul's dummy output rows carry d = E8 . S of the prev step)
            EB = sing.tile([P, P], bf16)
            nc.vector.memset(EB[:], 0.0)
            for g in range(16):
                nc.gpsimd.dma_start(EB[8 * g:8 * g + 7, 8 * g:8 * g + 8],
                                    EtA[:, :])
            ONEBD = sing.tile([P, 16], bf16)      # [k=(g,i<7), m=g'] = 1[g=g']
            nc.gpsimd.dma_start(ONEBD[:], c_onebd[:, :])
            TRI16 = sing.tile([16, 16], f32)      # [k, m] = 1[k < m]
            nc.gpsimd.dma_start(TRI16[:], c_tri16[:, :])
            ONES16 = sing.tile([16, 16], f32)
            nc.gpsimd.dma_start(ONES16[:], c_ones16[:, :])

            # ---------- feats DMA + emissions + XBAR transposes ----------
            featsb = sing.tile([BL, T, K], f32)
            # EFT[p=(g,j), tau, set, b] = exp(feats[b, (set*16+g)*C+tau, j]-G)
            EFT = sing.tile([P, C, NSET, BL], bf16)
            for s in range(NSET):
                ef2 = epool.tile([BL, C, 16, JP], bf16, tag="ef2")
                # dummy-state emission = 1.0 so matmul d-rows ride unscaled
                nc.vector.memset(ef2[:, :, :, 7:8], 1.0)
                for q in range(2):
                    cidx = s * 2 + q
                    t0 = cidx * FCH
                    eng = nc.sync if (cidx % 2 == 0) else nc.scalar
                    eng.dma_start(featsb[:, t0:t0 + FCH, :],
                                  feats[:, t0:t0 + FCH, :])
                    inap = featsb[:, t0:t0 + FCH, 0:KA] \
                        .rearrange("p (g tau) j -> p tau g j", tau=C)
                    nc.scalar.activation(ef2[:, :, 8 * q:8 * q + 8, 0:KA], inap,
                                         AF.Exp, bias=negG[:, 0:1])
                eng = nc.sync if (s % 2 == 0) else nc.scalar
                eng.dma_start(
                    EFT[:, :, s, :],
                    ef2[:].rearrange("p tau g j -> p (tau g j)"),
                    transpose=True)

            # ---------- gold: masks ----------
            maskb = sing.tile([BL, T], bf16)
            nc.vector.tensor_tensor(maskb[:], iota_t[:],
                                    lenf_sb[:].broadcast_to([BL, T]), op=OP.is_lt)
            tagp1m = sing.tile([BL, T], bf16)
            nc.vector.scalar_tensor_tensor(tagp1m[:], tagf_sb[:], 1.0, maskb[:],
                                           op0=OP.add, op1=OP.mult)

            # ---------- gold: trans part (2x deg-3 Horner + eqmul per row) ----
            acc = sing.tile([BL, 1], f32)
            nc.vector.memset(acc[:], 0.0)
            junk = sing.tile([BL, T], f32)
            prevs = tagp1m[:, 0:T - 1]
            curs = tagp1m[:, 1:T]
            for j in range(KA):
                cj = coefb[:, j * 7: j * 7 + 7]
                h1 = gpool.tile([BL, T], f32, tag="h1")
                nc.vector._custom_dve(OPS["ANT_H3_TOP"], out=h1[:, 0:T - 1],
                                      in0=prevs, in1=cj[:, 6:7],
                                      s0=cj[:, 3:4], s1=cj[:, 4:5],
                                      imm2=float(ch[j, 5]))
                h2 = gpool.tile([BL, T], f32, tag="h2")
                nc.vector._custom_dve(OPS["ANT_H3_STEP"], out=h2[:, 0:T - 1],
                                      in0=prevs, in1=h1[:, 0:T - 1],
                                      s0=cj[:, 0:1], s1=cj[:, 1:2],
                                      imm2=float(ch[j, 2]))
                acc2 = spool.tile([BL, 1], f32, tag="acc")
                nc.vector._custom_dve(OPS["ANT_EQMUL_ACC"], out=junk[:, 0:T - 1],
                                      in0=curs, in1=h2[:, 0:T - 1],
                                      s0=float(j + 1), s1=acc[:],
                                      accum_out=acc2[:])
                acc = acc2

            # ---------- forward: probe (taus C-4..C-1), unnormalized --------
            def eft_h(tau, h):
                return EFT[:, tau, 4 * h:4 * h + 4, :] \
                    .rearrange("p s b -> p (s b)")

            yp = [None, None]
            for h in range(2):
                y0 = cpool.tile([P, HC], bf16, tag=f"y{h}")
                nc.vector.tensor_tensor(
                    y0[:], eft_h(C - 4, h),
                    rsE[:, 0:1].broadcast_to([P, HC]), op=OP.mult)
                yp[h] = y0
            for tau in range(C - 3, C):
                for h in range(2):
                    mm = mmps.tile([P, HC], f32, tag=f"mm{h}")
                    nc.tensor.matmul(mm[:], EB[:], yp[h][:])
                    y_new = cpool.tile([P, HC], bf16, tag=f"y{h}")
                    nc.vector.tensor_tensor(y_new[:], mm[:], eft_h(tau, h),
                                            op=OP.mult)
                    yp[h] = y_new

            # probe block masses: lnm0[g, n] = ln(sum_{j<7} y[(g,j), n])
            lnm0 = sing.tile([16, COLS], f32)
            for h in range(2):
                mps = upps.tile([16, HC], f32, tag="m16")
                nc.tensor.matmul(mps[:], ONEBD[:], yp[h][:])
                nc.scalar.activation(lnm0[:, h * HC:(h + 1) * HC], mps[:],
                                     AF.Ln)

            # ---------- seeds (partition-shifted: DMA, engines can't) -------
            yin = sing.tile([P, COLS], bf16)
            nc.vector.memset(yin[0:8, 0:BL], 0.0)
            for h in range(2):
                nc.sync.dma_start(yin[8:P, h * HC:(h + 1) * HC],
                                  yp[h][0:P - 8, :])
            for s in range(1, NSET):
                sh, sc = (s - 1) // 4, ((s - 1) % 4) * BL
                nc.scalar.dma_start(yin[0:8, s * BL:(s + 1) * BL],
                                    yp[sh][P - 8:P, sc:sc + BL])

            # ---------- pass 2: two independent column-half chains ----------
            # y_new = (E y)*ef; EB dummy cols carry d = E8.S of the previous
            # step into rows (g,7) (ef pad = 1.0): hist region tau holds
            # d_{tau-1}; ghost step adds d_{C-1} at region C.
            y_last = [None, None]
            for tau in range(C):
                for h in range(2):
                    y_prev = yin[:, h * HC:(h + 1) * HC] if tau == 0 \
                        else y_last[h][:]
                    mm = mmps.tile([P, HC], f32, tag=f"mm{h}")
                    nc.tensor.matmul(mm[:], EB[:], y_prev)
                    y_new = cpool.tile([P, HC], bf16, tag=f"y{h}")
                    nc.vector.tensor_tensor(y_new[:], mm[:], eft_h(tau, h),
                                            op=OP.mult)
                    if tau == 0 and h == 0:
                        # block 0 exact init: S_0 = exp(trans[j,START])*ef_0
                        nc.vector.tensor_tensor(
                            y_new[0:KA, 0:BL],
                            E7e[:, 0:1].broadcast_to([KA, BL]),
                            EFT[0:KA, 0, 0, :], op=OP.mult)
                    drows = y_new[:].rearrange("(g j) n -> g j n", j=8)[:, 7, :]
                    eng = nc.sync if h == 0 else nc.scalar
                    eng.dma_start(
                        bass.AP(hist_d, tau * 16 * COLS + h * HC,
                                [[COLS, 16], [1, HC]]), drows)
                    y_last[h] = y_new
            # ghost step: d_{C-1} shows up in MM(C)'s dummy rows
            for h in range(2):
                mm = mmps.tile([P, HC], f32, tag=f"mm{h}")
                nc.tensor.matmul(mm[:], EB[:], y_last[h][:])
                yg = cpool.tile([P, HC], bf16, tag=f"y{h}")
                nc.scalar.copy(yg[:], mm[:])
                eng = nc.sync if h == 0 else nc.scalar
                gr = yg[:].rearrange("(g j) n -> g j n", j=8)[:, 7, :]
                eng.dma_start(
                    bass.AP(hist_d, C * 16 * COLS + h * HC,
                            [[COLS, 16], [1, HC]]), gr)

            # ---------- delta + exclusive prefix (Lam) ----------
            lam_sb = sing.tile([16, COLS], f32)
            for h in range(2):
                mEp = upps.tile([16, HC], f32, tag="m16")
                nc.tensor.matmul(mEp[:], ONEBD[:], y_last[h][:])
                nc.scalar.activation(lam_sb[:, h * HC:(h + 1) * HC], mEp[:],
                                     AF.Ln)
            dlt = sing.tile([16, COLS], f32)
            nc.vector.tensor_tensor(dlt[:], lam_sb[:], lnm0[:], op=OP.subtract)
            Lam_sb = sing.tile([16, COLS], f32)
            pairs = [(sp, s) for s in range(1, NSET) for sp in range(s)]
            for hh in range(2):
                Lps = upps.tile([16, HC], f32, tag=f"lps{hh}")
                hsl = slice(hh * HC, (hh + 1) * HC)
                hp = [(sp, s) for (sp, s) in pairs if s // 4 == hh]
                nc.tensor.matmul(Lps[:], TRI16[:], dlt[:, hsl],
                                 start=True, stop=(not hp),
                                 skip_group_check=True)
                for idx, (sp, s) in enumerate(hp):
                    c0 = (s % 4) * BL
                    nc.tensor.matmul(Lps[:, c0:c0 + BL], ONES16[:],
                                     dlt[:, sp * BL:(sp + 1) * BL],
                                     start=False, stop=(idx == len(hp) - 1),
                                     skip_group_check=True)
                nc.scalar.copy(Lam_sb[:, hsl], Lps[:])
            nc.sync.dma_start(bass.AP(offs_d, 0, [[COLS, 16], [1, COLS]]),
                              Lam_sb[:])

            # ---------- gold: feats part ----------
            facc = sing.tile([BL, 1], f32)
            nc.vector.memset(facc[:], 0.0)
            fa = facc
            for j in range(KA):
                fa2 = spool.tile([BL, 1], f32, tag="fa")
                nc.vector._custom_dve(OPS["ANT_EQMUL_ACC"], out=junk[:],
                                      in0=tagp1m[:, 0:T], in1=featsb[:, :, j],
                                      s0=float(j + 1), s1=fa[:],
                                      accum_out=fa2[:])
                fa = fa2
            facc = fa

            # ---------- gold: t0 term trans[tag_0, START] (coef row 8) -------
            x0 = tagp1m[:, 0:1]
            c8 = coefb[:, 56:63]
            p1 = spool.tile([BL, 1], f32, tag="p1")
            nc.vector._custom_dve(OPS["ANT_H3_TOP"], out=p1[:], in0=x0,
                                  in1=c8[:, 6:7], s0=c8[:, 3:4], s1=c8[:, 4:5],
                                  imm2=float(ch[8, 5]))
            t0p = spool.tile([BL, 1], f32, tag="t0p")
            nc.vector._custom_dve(OPS["ANT_H3_STEP"], out=t0p[:], in0=x0,
                                  in1=p1[:], s0=c8[:, 0:1], s1=c8[:, 1:2],
                                  imm2=float(ch[8, 2]))

            # ---------- gold: last term trans[STOP, tag_last] ----------
            idxT = spool.tile([BL, 1], i32, tag="idxT")
            nc.gpsimd.tensor_tensor(idxT[:], sTm1[:], leni_sb[:], op=OP.add)
            tgl = spool.tile([BL, 1], f32, tag="tgl")
            nc.gpsimd.indirect_dma_start(
                out=tgl[:], out_offset=None,
                in_=bass.AP(tagf, 0, [[1, BL * T], [1, 1]]),
                in_offset=bass.IndirectOffsetOnAxis(ap=idxT[:, 0:1], axis=0))
            f1 = spool.tile([BL, 1], f32, tag="f1")
            nc.vector.memset(f1[:], 1.0)
            xl = spool.tile([BL, 1], f32, tag="xl")
            nc.vector.tensor_tensor(xl[:], tgl[:], f1[:], op=OP.add)
            c7r = coefb[:, 49:56]
            q1 = spool.tile([BL, 1], f32, tag="q1")
            nc.vector._custom_dve(OPS["ANT_H3_TOP"], out=q1[:], in0=xl[:],
                                  in1=c7r[:, 6:7], s0=c7r[:, 3:4],
                                  s1=c7r[:, 4:5], imm2=float(ch[7, 5]))
            lastp = spool.tile([BL, 1], f32, tag="lastp")
            nc.vector._custom_dve(OPS["ANT_H3_STEP"], out=lastp[:], in0=xl[:],
                                  in1=q1[:], s0=c7r[:, 0:1], s1=c7r[:, 1:2],
                                  imm2=float(ch[7, 2]))

            # ---------- gather indices ----------
            # li0 = len-1; blk = li0>>4; tau = li0&15; s = blk>>4; g = blk&15
            # idxA = (tau+1)*16384 + g*1024 + s*128 + b
            # idxB = g*1024 + s*128 + b
            def gp_const(v):
                tl = spool.tile([BL, 1], i32, tag="ic")
                nc.gpsimd.memset(tl[:], v)
                return tl

            cm1 = gp_const(-1)
            c4i = gp_const(4)
            li0 = spool.tile([BL, 1], i32, tag="li0")
            nc.vector.tensor_tensor(li0[:], leni_sb[:], cm1[:], op=OP.add)
            blkt = spool.tile([BL, 1], i32, tag="blkt")
            nc.vector.tensor_tensor(blkt[:], li0[:], c4i[:],
                                    op=OP.logical_shift_right)
            tmp = spool.tile([BL, 1], i32, tag="tmp")
            nc.vector.tensor_tensor(tmp[:], blkt[:], c4i[:],
                                    op=OP.logical_shift_left)
            taut = spool.tile([BL, 1], i32, tag="taut")
            nc.vector.tensor_tensor(taut[:], li0[:], tmp[:], op=OP.subtract)
            st = spool.tile([BL, 1], i32, tag="st")
            nc.vector.tensor_tensor(st[:], blkt[:], c4i[:],
                                    op=OP.logical_shift_right)
            tmp2 = spool.tile([BL, 1], i32, tag="tmp2")
            nc.vector.tensor_tensor(tmp2[:], st[:], c4i[:],
                                    op=OP.logical_shift_left)
            gt = spool.tile([BL, 1], i32, tag="gt")
            nc.vector.tensor_tensor(gt[:], blkt[:], tmp2[:], op=OP.subtract)

            c14i = gp_const(14)
            c10i = gp_const(10)
            c7i = gp_const(7)
            idxB = spool.tile([BL, 1], i32, tag="idxB")
            nc.vector.tensor_tensor(idxB[:], gt[:], c10i[:],
                                    op=OP.logical_shift_left)
            ts = spool.tile([BL, 1], i32, tag="ts")
            nc.vector.tensor_tensor(ts[:], st[:], c7i[:],
                                    op=OP.logical_shift_left)
            nc.vector.tensor_tensor(idxB[:], idxB[:], ts[:], op=OP.add)
            nc.vector.tensor_tensor(idxB[:], idxB[:], b32[:], op=OP.add)
            c16k = gp_const(16 * COLS)
            idxA = spool.tile([BL, 1], i32, tag="idxA")
            nc.vector.tensor_tensor(idxA[:], taut[:], c14i[:],
                                    op=OP.logical_shift_left)
            nc.vector.tensor_tensor(idxA[:], idxA[:], idxB[:], op=OP.add)
            nc.vector.tensor_tensor(idxA[:], idxA[:], c16k[:], op=OP.add)

            # ---------- gathers ----------
            dg = spool.tile([BL, 1], bf16, tag="dg")
            nc.gpsimd.indirect_dma_start(
                out=dg[:], out_offset=None,
                in_=bass.AP(hist_d, 0, [[1, (C + 1) * 16 * COLS], [1, 1]]),
                in_offset=bass.IndirectOffsetOnAxis(ap=idxA[:, 0:1], axis=0))
            offg = spool.tile([BL, 1], f32, tag="offg")
            nc.gpsimd.indirect_dma_start(
                out=offg[:], out_offset=None,
                in_=bass.AP(offs_d, 0, [[1, 16 * COLS], [1, 1]]),
                in_offset=bass.IndirectOffsetOnAxis(ap=idxB[:, 0:1], axis=0))

            # ---------- finalize ----------
            lnv = spool.tile([BL, 1], f32, tag="lnv")
            nc.scalar.activation(lnv[:], dg[:], AF.Ln)
            fwd1 = spool.tile([BL, 1], f32, tag="fwd1")
            nc.vector.tensor_tensor(fwd1[:], lnv[:], offg[:], op=OP.add)
            fwd2 = spool.tile([BL, 1], f32, tag="fwd2")
            nc.vector.scalar_tensor_tensor(fwd2[:], lenf_sb[:], G, fwd1[:],
                                           op0=OP.mult, op1=OP.add)
            g2 = spool.tile([BL, 1], f32, tag="g2")
            nc.vector.tensor_tensor(g2[:], t0p[:], lastp[:], op=OP.add)
            g3 = spool.tile([BL, 1], f32, tag="g3")
            nc.vector.tensor_tensor(g3[:], acc[:], g2[:], op=OP.add)
            g4 = spool.tile([BL, 1], f32, tag="g4")
            nc.vector.tensor_tensor(g4[:], g3[:], facc[:], op=OP.add)
            res = spool.tile([BL, 1], f32, tag="res")
            nc.vector.tensor_tensor(res[:], fwd2[:], g4[:], op=OP.subtract)
            nc.sync.dma_start(outv[:, :], res[:])

    nc.finalize()
    return nc


def _coefs(transitions):
    tr = np.asarray(transitions, np.float64)
    V = np.vander(np.arange(1, 8, dtype=np.float64), 7, increasing=True)
    rows = [np.linalg.solve(V, tr[j, 0:7]) for j in range(7)]
    rows.append(np.linalg.solve(V, tr[STOP, 0:7]))
    rows.append(np.linalg.solve(V, tr[0:7, START]))
    return np.stack(rows).astype(np.float32)


def kernel(feats, transitions, tags, lengths):
    feats = np.ascontiguousarray(np.asarray(feats, dtype=np.float32))
    transitions = np.ascontiguousarray(np.asarray(transitions, dtype=np.float32))
    tags_f = np.ascontiguousarray(np.asarray(tags).astype(np.float32))
    len_f = np.ascontiguousarray(np.asarray(lengths).astype(np.float32).reshape(B, 1))
    len_i = np.ascontiguousarray(np.asarray(lengths).astype(np.int32).reshape(B, 1))
    coefs = np.ascontiguousarray(_coefs(transitions))

    key = ("nc", transitions.tobytes())
    if key not in _CACHE:
        _CACHE[key] = _build_bass(coefs.astype(np.float64))
    nc = _CACHE[key]

    from concourse.bass_utils import run_bass_kernel_spmd

    in_maps = []
    for c in range(NCORES):
        sl = slice(c * BL, (c + 1) * BL)
        in_maps.append({
            "feats": feats[sl],
            "tagf": tags_f[sl],
            "lenf": len_f[sl],
            "leni": len_i[sl],
            "trans": transitions,
            "coefs": coefs,
        })
    r = run_bass_kernel_spmd(nc, in_maps, core_ids=list(range(NCORES)),
                             trace=TRACE)
    if TRACE:
        _CACHE["last_result"] = r
    per_seq = np.concatenate([m["outv"].reshape(BL) for m in r.results])
    return np.float32(per_seq.mean(dtype=np.float64))


# revision 37
# speedup vs baseline: 1.0440x; 1.0440x over previous
"""BERT_BiLSTM_CRF loss (CRF NLL) Trainium2 kernel — TensorE-forward version.

Self-contained: kernel(**inputs) takes FULL inputs, shards batch across 8
NeuronCores (128 seqs/core), returns the scalar mean loss.

Forward: the CRF recurrence S_t = diag(ef_t) E S_{t-1} (exp domain,
G-centered emissions) runs on the TensorEngine in a transposed layout:
partitions = 16 groups x 8 states (7 real + 1 dummy), columns = sets x 128
seqs; blk = set*16+g gives NBLK=128 time blocks of C=16. Per step and
column-half: one block-diagonal [128x128]x[128,512] matmul + one DVE mult
by transposed emissions EFT (built by DMA-XBAR transposes of exp(feats-G)).
The stationary's dummy columns hold E8 = exp(trans[STOP,:]) and the dummy
emission is 1.0, so each matmul's dummy output rows carry the previous
step's final-score dot d = E8 . S — written per step to DRAM (16 rows via
partition-strided DMA). Probe pass (last 4 taus, uniform start) seeds all
blocks (unnormalized); per-block delta = ln(end mass) - ln(probe mass)
telescopes into the per-seq offset via triangular 16x16 matmul prefix.
fwd = ln(d*) + Lam* + G*len, d*/Lam* fetched by indirect DMA at len-1.
Validated vs the exact reference in numpy (bf16 chain: loss rel 8e-7).

Gold scores: sum_t feat[t,tag_t] via eq-mask/mult/reduce on GPSIMD;
sum_t trans[tag_t,tag_{t-1}] via exact degree-6 Horner polynomials
(host-solved Vandermonde coefficients; two degree-3 custom DVE ops per row,
quadratic coeffs inlined as immediates) + eq-masked accumulation per row.
"""

import numpy as np

B, T, K = 1024, 2048, 9
NCORES = 8
BL = B // NCORES          # sequences per core (=128 partitions)
KA = 7                    # active states
JP = 8                    # padded state dim (j=7 is the d-carrier dummy)
START, STOP = 7, 8
G = 2.4                   # per-step log growth centering
C = 16                    # block length
NBLK = T // C             # 128 blocks
NSET = NBLK // 16         # 8 sets; blk = set*16 + g
P = 128                   # partitions in transposed layout: p = g*8 + j
COLS = NSET * BL          # 1024 columns: n = set*128 + b
HC = COLS // 2            # column half (4 sets)
FCH = T // 16             # feats DMA chunk (128 timesteps = 8 blocks)

_CACHE = {}
TRACE = False


def _register_dve_ops():
    import concourse.dve_ops as DO
    from concourse.dve_spec import Spec, Src0, Src1, C0, C1, C2, C3, eq, \
        lower, _spill_c3_to_src1
    from concourse.dve_uop import DveOpSpec
    from concourse.dve_spec import AluOp as SAluOp

    existing = {o.name: o for o in DO.OPS}

    def mk(name, spec, subdim=False):
        if name in existing:
            return existing[name]
        op = DO.DveOp(name, spec, subdim, uops_sha={})
        DO.OPS.append(op)
        DO.CUSTOM_DVE_SPECS[name] = spec
        DO._SUB_OPCODE_FOR_NAME[name] = DO._CUSTOM_DVE_ROW_BASE + len(DO.OPS) - 1
        for ver in ("v3", "v4"):
            r = DveOpSpec(name=name, opcode=DO.get_dve_sub_opcode(name),
                          uops=lower(spec, ver=ver), rd1_en=DO.has_src1(spec))
            op.uops_sha[ver] = r.sha(ver)
        return op

    def _eqmul_ref(in0, in1, s0, s1, imm2):
        out = (np.asarray(in0, np.float32) == s0) * np.asarray(in1, np.float32)
        acc = (s1 if isinstance(s1, float) else np.asarray(s1, np.float32)) \
            + out.sum(axis=1, keepdims=True)
        return out, acc

    def _h3top_ref(in0, in1, s0, s1, imm2):
        x = np.asarray(in0, np.float32)
        c3 = np.asarray(in1, np.float32)
        return s0 + x * (s1 + x * (imm2 + x * c3))

    def _h3step_ref(in0, in1, s0, s1, imm2):
        x = np.asarray(in0, np.float32)
        h = np.asarray(in1, np.float32)
        return s0 + x * (s1 + x * (imm2 + x * h))

    ops = {}
    # accum_out = s1 + sum_n (Src0==s0)*Src1
    ops["ANT_EQMUL_ACC"] = mk(
        "ANT_EQMUL_ACC",
        Spec(body=eq(Src0, C0) * Src1, accum=SAluOp.ADD, accum_init=C1,
             reference=_eqmul_ref))
    # out = s0 + x*(s1 + x*(imm2 + x*c3)) with c3 spilled to in1 (elem 0)
    ops["ANT_H3_TOP"] = mk(
        "ANT_H3_TOP",
        Spec(body=_spill_c3_to_src1(C0 + Src0 * (C1 + Src0 * (C2 + Src0 * C3))),
             reference=_h3top_ref))
    # out = s0 + x*(s1 + x*(imm2 + x*Src1))
    ops["ANT_H3_STEP"] = mk(
        "ANT_H3_STEP",
        Spec(body=C0 + Src0 * (C1 + Src0 * (C2 + Src0 * Src1)),
             reference=_h3step_ref))
    return ops


def _build_bass(coefs_host):
    import concourse.bass as bass
    import concourse.bacc as bacc
    import concourse.tile as tile
    import concourse.mybir as mybir

    OPS = _register_dve_ops()

    f32 = mybir.dt.float32
    bf16 = mybir.dt.bfloat16
    i32 = mybir.dt.int32
    AX = mybir.AxisListType
    OP = mybir.AluOpType
    AF = mybir.ActivationFunctionType

    nc = bacc.Bacc()

    feats = nc.dram_tensor("feats", [BL, T, K], f32, kind="ExternalInput")
    tagf = nc.dram_tensor("tagf", [BL, T], f32, kind="ExternalInput")
    lenf = nc.dram_tensor("lenf", [BL, 1], f32, kind="ExternalInput")
    leni = nc.dram_tensor("leni", [BL, 1], i32, kind="ExternalInput")
    trans = nc.dram_tensor("trans", [K, K], f32, kind="ExternalInput")
    # Horner coefficients (host Vandermonde): rows 0..6 = trans[j, x-1],
    # row 7 = trans[STOP, x-1], row 8 = trans[x-1, START]; all in x=tag+1.
    coefs = nc.dram_tensor("coefs", [9, 7], f32, kind="ExternalInput")
    outv = nc.dram_tensor("outv", [BL, 1], f32, kind="ExternalOutput")

    hist_d = nc.dram_tensor("hist_d", [(C + 1) * 16 * COLS, 1], bf16)
    offs_d = nc.dram_tensor("offs_d", [16 * COLS, 1], f32)

    iota_t_np = np.arange(T, dtype=np.float32).reshape(1, T)
    c_iota_t = nc.inline_tensor(iota_t_np, "c_iota_t")
    c_b32 = nc.inline_tensor(np.arange(BL, dtype=np.int32).reshape(BL, 1),
                             "c_b32")
    c_sTm1 = nc.inline_tensor((np.arange(BL, dtype=np.int64) * T - 1)
                              .astype(np.int32).reshape(BL, 1), "c_sTm1")
    onebd_np = np.zeros((P, 16), np.float32)
    for g in range(16):
        onebd_np[8 * g:8 * g + 7, g] = 1.0   # exclude dummy row j=7
    c_onebd = nc.inline_tensor(onebd_np, "c_onebd")
    tri16_np = np.triu(np.ones((16, 16), np.float32), 1)  # [k,m]=1 iff k<m
    c_tri16 = nc.inline_tensor(tri16_np, "c_tri16")
    c_ones16 = nc.inline_tensor(np.ones((16, 16), np.float32), "c_ones16")

    ch = coefs_host  # [9, 7] float, for inline immediates (quadratic coefs)

    with tile.TileContext(nc) as tc:
        import contextlib
        ctx = contextlib.ExitStack()
        with ctx:
            sing = ctx.enter_context(tc.tile_pool(name="sing", bufs=1))
            epool = ctx.enter_context(tc.tile_pool(name="epool", bufs=5))
            cpool = ctx.enter_context(tc.tile_pool(name="cpool", bufs=3))
            gpool = ctx.enter_context(tc.tile_pool(name="gpool", bufs=1))
            gq = ctx.enter_context(tc.tile_pool(name="gq", bufs=1))
            spool = ctx.enter_context(tc.tile_pool(name="spool", bufs=4))
            mmps = ctx.enter_context(
                tc.tile_pool(name="mmps", bufs=2, space="PSUM"))
            upps = ctx.enter_context(
                tc.tile_pool(name="upps", bufs=1, space="PSUM"))

            # ---------- tiny constants ----------
            negG = sing.tile([BL, 1], f32)
            nc.gpsimd.memset(negG[:], -G)
            coefb = sing.tile([BL, 63], f32)
            nc.gpsimd.dma_start(coefb[:], bass.AP(coefs, 0, [[0, BL], [1, 63]]))
            iota_t = sing.tile([BL, T], f32)
            nc.gpsimd.dma_start(iota_t[:], bass.AP(c_iota_t, 0, [[0, BL], [1, T]]))
            b32 = sing.tile([BL, 1], i32)
            nc.gpsimd.dma_start(b32[:], c_b32[:, :])
            sTm1 = sing.tile([BL, 1], i32)
            nc.gpsimd.dma_start(sTm1[:], c_sTm1[:, :])
            lenf_sb = sing.tile([BL, 1], f32)
            nc.gpsimd.dma_start(lenf_sb[:], lenf[:, :])
            leni_sb = sing.tile([BL, 1], i32)
            nc.gpsimd.dma_start(leni_sb[:], leni[:, :])

            # tags: needed early for gold
            tagf_sb = sing.tile([BL, T], f32)
            nc.sync.dma_start(tagf_sb[:], tagf[:, :])

            # E^T extended tile: EtA[i, j<7] = exp(trans[j, i]),
            # EtA[i, 7] = exp(trans[STOP, i])  (final-score column)
            t7x = sing.tile([7, 8], f32)
            nc.gpsimd.dma_start(t7x[:, 0:7], bass.AP(trans, 0, [[1, 7], [9, 7]]))
            nc.gpsimd.dma_start(t7x[:, 7:8],
                                bass.AP(trans, STOP * K, [[1, 7], [1, 1]]))
            EtA = sing.tile([7, 8], bf16)
            nc.scalar.activation(EtA[:], t7x[:], AF.Exp)
            # E7[j] = exp(trans[j, START]) on partitions j
            t7b = sing.tile([7, 1], f32)
            nc.gpsimd.dma_start(t7b[:], bass.AP(trans, START, [[9, 7], [1, 1]]))
            E7e = sing.tile([7, 1], f32)
            nc.scalar.activation(E7e[:], t7b[:], AF.Exp)
            # rsE[j] = sum_i E[j,i] (no matmul: row-major E tile + reduce)
            t7r = sing.tile([7, 7], f32)
            nc.gpsimd.dma_start(t7r[:], bass.AP(trans, 0, [[9, 7], [1, 7]]))
            Ete = sing.tile([7, 7], f32)
            nc.scalar.activation(Ete[:], t7r[:], AF.Exp)
            rs7 = sing.tile([7, 1], f32)
            nc.vector.tensor_reduce(out=rs7[:], in_=Ete[:], axis=AX.X, op=OP.add)
            rsE = sing.tile([P, 1], f32)
            nc.gpsimd.memset(rsE[:], 0.0)
            for g in range(16):
                nc.gpsimd.dma_start(rsE[8 * g:8 * g + 7, :], rs7[:, :])

            # ---------- stationary matrices ----------
            # EB blockdiag: col (g,j<7) = E^T block; col (g,7) = E8 (so each
            # matmul's dummy output rows carry d = E8 . S of the prev step)
            EB = sing.tile([P, P], bf16)
            nc.vector.memset(EB[:], 0.0)
            for g in range(16):
                nc.gpsimd.dma_start(EB[8 * g:8 * g + 7, 8 * g:8 * g + 8],
                                    EtA[:, :])
            ONEBD = sing.tile([P, 16], bf16)      # [k=(g,i<7), m=g'] = 1[g=g']
            nc.gpsimd.dma_start(ONEBD[:], c_onebd[:, :])
            TRI16 = sing.tile([16, 16], f32)      # [k, m] = 1[k < m]
            nc.gpsimd.dma_start(TRI16[:], c_tri16[:, :])
            ONES16 = sing.tile([16, 16], f32)
            nc.gpsimd.dma_start(ONES16[:], c_ones16[:, :])

            # ---------- feats DMA + emissions + XBAR transposes ----------
            featsb = sing.tile([BL, T, K], f32)
            # EFT[p=(g,j), tau, set, b] = exp(feats[b, (set*16+g)*C+tau, j]-G)
            EFT = sing.tile([P, C, NSET, BL], bf16)
            # all chunk DMAs first (no interleaved waits -> queues stream)
            for cidx in range(16):
                t0 = cidx * FCH
                eng = nc.sync if (cidx % 2 == 0) else nc.scalar
                eng.dma_start(featsb[:, t0:t0 + FCH, :],
                              feats[:, t0:t0 + FCH, :])
            for s in range(NSET):
                ef2 = epool.tile([BL, C, 16, JP], bf16, tag="ef2")
                # dummy-state emission = 1.0 so matmul d-rows ride unscaled
                nc.gpsimd.memset(ef2[:, :, :, 7:8], 1.0)
                for q in range(2):
                    cidx = s * 2 + q
                    t0 = cidx * FCH
                    inap = featsb[:, t0:t0 + FCH, 0:KA] \
                        .rearrange("p (g tau) j -> p tau g j", tau=C)
                    nc.scalar.activation(ef2[:, :, 8 * q:8 * q + 8, 0:KA], inap,
                                         AF.Exp, bias=negG[:, 0:1])
                eng = nc.sync if (s % 2 == 0) else nc.scalar
                eng.dma_start(
                    EFT[:, :, s, :],
                    ef2[:].rearrange("p tau g j -> p (tau g j)"),
                    transpose=True)

            # ---------- gold: masks ----------
            maskb = sing.tile([BL, T], bf16)
            nc.vector.tensor_tensor(maskb[:], iota_t[:],
                                    lenf_sb[:].broadcast_to([BL, T]), op=OP.is_lt)
            tagp1m = sing.tile([BL, T], bf16)
            nc.vector.scalar_tensor_tensor(tagp1m[:], tagf_sb[:], 1.0, maskb[:],
                                           op0=OP.add, op1=OP.mult)

            # ---------- gold: trans part (2x deg-3 Horner + eqmul per row) ----
            acc = sing.tile([BL, 1], f32)
            nc.vector.memset(acc[:], 0.0)
            junk = sing.tile([BL, T], f32)
            prevs = tagp1m[:, 0:T - 1]
            curs = tagp1m[:, 1:T]
            for j in range(KA):
                cj = coefb[:, j * 7: j * 7 + 7]
                h1 = gpool.tile([BL, T], f32, tag="h1")
                nc.vector._custom_dve(OPS["ANT_H3_TOP"], out=h1[:, 0:T - 1],
                                      in0=prevs, in1=cj[:, 6:7],
                                      s0=cj[:, 3:4], s1=cj[:, 4:5],
                                      imm2=float(ch[j, 5]))
                h2 = gpool.tile([BL, T], f32, tag="h2")
                nc.vector._custom_dve(OPS["ANT_H3_STEP"], out=h2[:, 0:T - 1],
                                      in0=prevs, in1=h1[:, 0:T - 1],
                                      s0=cj[:, 0:1], s1=cj[:, 1:2],
                                      imm2=float(ch[j, 2]))
                acc2 = spool.tile([BL, 1], f32, tag="acc")
                nc.vector._custom_dve(OPS["ANT_EQMUL_ACC"], out=junk[:, 0:T - 1],
                                      in0=curs, in1=h2[:, 0:T - 1],
                                      s0=float(j + 1), s1=acc[:],
                                      accum_out=acc2[:])
                acc = acc2

            # ---------- forward: probe (taus C-4..C-1), unnormalized --------
            def eft_h(tau, h):
                return EFT[:, tau, 4 * h:4 * h + 4, :] \
                    .rearrange("p s b -> p (s b)")

            yp = [None, None]
            for h in range(2):
                y0 = cpool.tile([P, HC], bf16, tag=f"y{h}")
                nc.vector.tensor_tensor(
                    y0[:], eft_h(C - 4, h),
                    rsE[:, 0:1].broadcast_to([P, HC]), op=OP.mult)
                yp[h] = y0
            for tau in range(C - 3, C):
                for h in range(2):
                    mm = mmps.tile([P, HC], f32, tag=f"mm{h}")
                    nc.tensor.matmul(mm[:], EB[:], yp[h][:])
                    y_new = cpool.tile([P, HC], bf16, tag=f"y{h}")
                    nc.vector.tensor_tensor(y_new[:], mm[:], eft_h(tau, h),
                                            op=OP.mult)
                    yp[h] = y_new

            # probe block masses: lnm0[g, n] = ln(sum_{j<7} y[(g,j), n])
            lnm0 = sing.tile([16, COLS], f32)
            for h in range(2):
                mps = upps.tile([16, HC], f32, tag="m16")
                nc.tensor.matmul(mps[:], ONEBD[:], yp[h][:])
                nc.scalar.activation(lnm0[:, h * HC:(h + 1) * HC], mps[:],
                                     AF.Ln)

            # ---------- seeds (partition-shifted: DMA, engines can't) -------
            yin = sing.tile([P, COLS], bf16)
            nc.vector.memset(yin[0:8, 0:BL], 0.0)
            for h in range(2):
                nc.sync.dma_start(yin[8:P, h * HC:(h + 1) * HC],
                                  yp[h][0:P - 8, :])
            for s in range(1, NSET):
                sh, sc = (s - 1) // 4, ((s - 1) % 4) * BL
                nc.scalar.dma_start(yin[0:8, s * BL:(s + 1) * BL],
                                    yp[sh][P - 8:P, sc:sc + BL])

            # ---------- pass 2: two independent column-half chains ----------
            # y_new = (E y)*ef; EB dummy cols carry d = E8.S of the previous
            # step into rows (g,7) (ef pad = 1.0): hist region tau holds
            # d_{tau-1}; ghost step adds d_{C-1} at region C.
            y_last = [None, None]
            for tau in range(C):
                for h in range(2):
                    y_prev = yin[:, h * HC:(h + 1) * HC] if tau == 0 \
                        else y_last[h][:]
                    mm = mmps.tile([P, HC], f32, tag=f"mm{h}")
                    nc.tensor.matmul(mm[:], EB[:], y_prev)
                    y_new = cpool.tile([P, HC], bf16, tag=f"y{h}")
                    nc.vector.tensor_tensor(y_new[:], mm[:], eft_h(tau, h),
                                            op=OP.mult)
                    if tau == 0 and h == 0:
                        # block 0 exact init: S_0 = exp(trans[j,START])*ef_0
                        nc.vector.tensor_tensor(
                            y_new[0:KA, 0:BL],
                            E7e[:, 0:1].broadcast_to([KA, BL]),
                            EFT[0:KA, 0, 0, :], op=OP.mult)
                    drows = y_new[:].rearrange("(g j) n -> g j n", j=8)[:, 7, :]
                    eng = nc.sync if h == 0 else nc.scalar
                    eng.dma_start(
                        bass.AP(hist_d, tau * 16 * COLS + h * HC,
                                [[COLS, 16], [1, HC]]), drows)
                    y_last[h] = y_new
            # ghost step: d_{C-1} shows up in MM(C)'s dummy rows
            for h in range(2):
                mm = mmps.tile([P, HC], f32, tag=f"mm{h}")
                nc.tensor.matmul(mm[:], EB[:], y_last[h][:])
                yg = cpool.tile([P, HC], bf16, tag=f"y{h}")
                nc.scalar.copy(yg[:], mm[:])
                eng = nc.sync if h == 0 else nc.scalar
                gr = yg[:].rearrange("(g j) n -> g j n", j=8)[:, 7, :]
                eng.dma_start(
                    bass.AP(hist_d, C * 16 * COLS + h * HC,
                            [[COLS, 16], [1, HC]]), gr)

            # ---------- delta + exclusive prefix (Lam) ----------
            lam_sb = sing.tile([16, COLS], f32)
            for h in range(2):
                mEp = upps.tile([16, HC], f32, tag="m16")
                nc.tensor.matmul(mEp[:], ONEBD[:], y_last[h][:])
                nc.scalar.activation(lam_sb[:, h * HC:(h + 1) * HC], mEp[:],
                                     AF.Ln)
            dlt = sing.tile([16, COLS], f32)
            nc.vector.tensor_tensor(dlt[:], lam_sb[:], lnm0[:], op=OP.subtract)
            Lam_sb = sing.tile([16, COLS], f32)
            pairs = [(sp, s) for s in range(1, NSET) for sp in range(s)]
            for hh in range(2):
                Lps = upps.tile([16, HC], f32, tag=f"lps{hh}")
                hsl = slice(hh * HC, (hh + 1) * HC)
                hp = [(sp, s) for (sp, s) in pairs if s // 4 == hh]
                nc.tensor.matmul(Lps[:], TRI16[:], dlt[:, hsl],
                                 start=True, stop=(not hp),
                                 skip_group_check=True)
                for idx, (sp, s) in enumerate(hp):
                    c0 = (s % 4) * BL
                    nc.tensor.matmul(Lps[:, c0:c0 + BL], ONES16[:],
                                     dlt[:, sp * BL:(sp + 1) * BL],
                                     start=False, stop=(idx == len(hp) - 1),
                                     skip_group_check=True)
                nc.scalar.copy(Lam_sb[:, hsl], Lps[:])
            nc.sync.dma_start(bass.AP(offs_d, 0, [[COLS, 16], [1, COLS]]),
                              Lam_sb[:])

            # ---------- gold: feats part ----------
            facc = sing.tile([BL, 1], f32)
            nc.vector.memset(facc[:], 0.0)
            fa = facc
            for j in range(KA):
                fa2 = spool.tile([BL, 1], f32, tag="fa")
                nc.vector._custom_dve(OPS["ANT_EQMUL_ACC"], out=junk[:],
                                      in0=tagp1m[:, 0:T], in1=featsb[:, :, j],
                                      s0=float(j + 1), s1=fa[:],
                                      accum_out=fa2[:])
                fa = fa2
            facc = fa

            # ---------- gold: t0 term trans[tag_0, START] (coef row 8) -------
            x0 = tagp1m[:, 0:1]
            c8 = coefb[:, 56:63]
            p1 = spool.tile([BL, 1], f32, tag="p1")
            nc.vector._custom_dve(OPS["ANT_H3_TOP"], out=p1[:], in0=x0,
                                  in1=c8[:, 6:7], s0=c8[:, 3:4], s1=c8[:, 4:5],
                                  imm2=float(ch[8, 5]))
            t0p = spool.tile([BL, 1], f32, tag="t0p")
            nc.vector._custom_dve(OPS["ANT_H3_STEP"], out=t0p[:], in0=x0,
                                  in1=p1[:], s0=c8[:, 0:1], s1=c8[:, 1:2],
                                  imm2=float(ch[8, 2]))

            # ---------- gold: last term trans[STOP, tag_last] ----------
            idxT = spool.tile([BL, 1], i32, tag="idxT")
            nc.gpsimd.tensor_tensor(idxT[:], sTm1[:], leni_sb[:], op=OP.add)
            tgl = spool.tile([BL, 1], f32, tag="tgl")
            nc.gpsimd.indirect_dma_start(
                out=tgl[:], out_offset=None,
                in_=bass.AP(tagf, 0, [[1, BL * T], [1, 1]]),
                in_offset=bass.IndirectOffsetOnAxis(ap=idxT[:, 0:1], axis=0))
            f1 = spool.tile([BL, 1], f32, tag="f1")
            nc.vector.memset(f1[:], 1.0)
            xl = spool.tile([BL, 1], f32, tag="xl")
            nc.vector.tensor_tensor(xl[:], tgl[:], f1[:], op=OP.add)
            c7r = coefb[:, 49:56]
            q1 = spool.tile([BL, 1], f32, tag="q1")
            nc.vector._custom_dve(OPS["ANT_H3_TOP"], out=q1[:], in0=xl[:],
                                  in1=c7r[:, 6:7], s0=c7r[:, 3:4],
                                  s1=c7r[:, 4:5], imm2=float(ch[7, 5]))
            lastp = spool.tile([BL, 1], f32, tag="lastp")
            nc.vector._custom_dve(OPS["ANT_H3_STEP"], out=lastp[:], in0=xl[:],
                                  in1=q1[:], s0=c7r[:, 0:1], s1=c7r[:, 1:2],
                                  imm2=float(ch[7, 2]))

            # ---------- gather indices ----------
            # li0 = len-1; blk = li0>>4; tau = li0&15; s = blk>>4; g = blk&15
            # idxA = (tau+1)*16384 + g*1024 + s*128 + b
            # idxB = g*1024 + s*128 + b
            def gp_const(v):
                tl = spool.tile([BL, 1], i32, tag="ic")
                nc.gpsimd.memset(tl[:], v)
                return tl

            cm1 = gp_const(-1)
            c4i = gp_const(4)
            li0 = spool.tile([BL, 1], i32, tag="li0")
            nc.vector.tensor_tensor(li0[:], leni_sb[:], cm1[:], op=OP.add)
            blkt = spool.tile([BL, 1], i32, tag="blkt")
            nc.vector.tensor_tensor(blkt[:], li0[:], c4i[:],
                                    op=OP.logical_shift_right)
            tmp = spool.tile([BL, 1], i32, tag="tmp")
            nc.vector.tensor_tensor(tmp[:], blkt[:], c4i[:],
                                    op=OP.logical_shift_left)
            taut = spool.tile([BL, 1], i32, tag="taut")
            nc.vector.tensor_tensor(taut[:], li0[:], tmp[:], op=OP.subtract)
            st = spool.tile([BL, 1], i32, tag="st")
            nc.vector.tensor_tensor(st[:], blkt[:], c4i[:],
                                    op=OP.logical_shift_right)
            tmp2 = spool.tile([BL, 1], i32, tag="tmp2")
            nc.vector.tensor_tensor(tmp2[:], st[:], c4i[:],
                                    op=OP.logical_shift_left)
            gt = spool.tile([BL, 1], i32, tag="gt")
            nc.vector.tensor_tensor(gt[:], blkt[:], tmp2[:], op=OP.subtract)

            c14i = gp_const(14)
            c10i = gp_const(10)
            c7i = gp_const(7)
            idxB = spool.tile([BL, 1], i32, tag="idxB")
            nc.vector.tensor_tensor(idxB[:], gt[:], c10i[:],
                                    op=OP.logical_shift_left)
            ts = spool.tile([BL, 1], i32, tag="ts")
            nc.vector.tensor_tensor(ts[:], st[:], c7i[:],
                                    op=OP.logical_shift_left)
            nc.vector.tensor_tensor(idxB[:], idxB[:], ts[:], op=OP.add)
            nc.vector.tensor_tensor(idxB[:], idxB[:], b32[:], op=OP.add)
            c16k = gp_const(16 * COLS)
            idxA = spool.tile([BL, 1], i32, tag="idxA")
            nc.vector.tensor_tensor(idxA[:], taut[:], c14i[:],
                                    op=OP.logical_shift_left)
            nc.vector.tensor_tensor(idxA[:], idxA[:], idxB[:], op=OP.add)
            nc.vector.tensor_tensor(idxA[:], idxA[:], c16k[:], op=OP.add)

            # ---------- gathers ----------
            dg = spool.tile([BL, 1], bf16, tag="dg")
            nc.gpsimd.indirect_dma_start(
                out=dg[:], out_offset=None,
                in_=bass.AP(hist_d, 0, [[1, (C + 1) * 16 * COLS], [1, 1]]),
                in_offset=bass.IndirectOffsetOnAxis(ap=idxA[:, 0:1], axis=0))
            offg = spool.tile([BL, 1], f32, tag="offg")
            nc.gpsimd.indirect_dma_start(
                out=offg[:], out_offset=None,
                in_=bass.AP(offs_d, 0, [[1, 16 * COLS], [1, 1]]),
                in_offset=bass.IndirectOffsetOnAxis(ap=idxB[:, 0:1], axis=0))

            # ---------- finalize ----------
            lnv = spool.tile([BL, 1], f32, tag="lnv")
            nc.scalar.activation(lnv[:], dg[:], AF.Ln)
            fwd1 = spool.tile([BL, 1], f32, tag="fwd1")
            nc.vector.tensor_tensor(fwd1[:], lnv[:], offg[:], op=OP.add)
            fwd2 = spool.tile([BL, 1], f32, tag="fwd2")
            nc.vector.scalar_tensor_tensor(fwd2[:], lenf_sb[:], G, fwd1[:],
                                           op0=OP.mult, op1=OP.add)
            g2 = spool.tile([BL, 1], f32, tag="g2")
            nc.vector.tensor_tensor(g2[:], t0p[:], lastp[:], op=OP.add)
            g3 = spool.tile([BL, 1], f32, tag="g3")
            nc.vector.tensor_tensor(g3[:], acc[:], g2[:], op=OP.add)
            g4 = spool.tile([BL, 1], f32, tag="g4")
            nc.vector.tensor_tensor(g4[:], g3[:], facc[:], op=OP.add)
            res = spool.tile([BL, 1], f32, tag="res")
            nc.vector.tensor_tensor(res[:], fwd2[:], g4[:], op=OP.subtract)
            nc.sync.dma_start(outv[:, :], res[:])

    nc.finalize()
    return nc


def _coefs(transitions):
    tr = np.asarray(transitions, np.float64)
    V = np.vander(np.arange(1, 8, dtype=np.float64), 7, increasing=True)
    rows = [np.linalg.solve(V, tr[j, 0:7]) for j in range(7)]
    rows.append(np.linalg.solve(V, tr[STOP, 0:7]))
    rows.append(np.linalg.solve(V, tr[0:7, START]))
    return np.stack(rows).astype(np.float32)


def kernel(feats, transitions, tags, lengths):
    feats = np.ascontiguousarray(np.asarray(feats, dtype=np.float32))
    transitions = np.ascontiguousarray(np.asarray(transitions, dtype=np.float32))
    tags_f = np.ascontiguousarray(np.asarray(tags).astype(np.float32))
    len_f = np.ascontiguousarray(np.asarray(lengths).astype(np.float32).reshape(B, 1))
    len_i = np.ascontiguousarray(np.asarray(lengths).astype(np.int32).reshape(B, 1))
    coefs = np.ascontiguousarray(_coefs(transitions))

    key = ("nc", transitions.tobytes())
    if key not in _CACHE:
        _CACHE[key] = _build_bass(coefs.astype(np.float64))
    nc = _CACHE[key]

    from concourse.bass_utils import run_bass_kernel_spmd

    in_maps = []
    for c in range(NCORES):
        sl = slice(c * BL, (c + 1) * BL)
        in_maps.append({
            "feats": feats[sl],
            "tagf": tags_f[sl],
            "lenf": len_f[sl],
            "leni": len_i[sl],
            "trans": transitions,
            "coefs": coefs,
        })
    r = run_bass_kernel_spmd(nc, in_maps, core_ids=list(range(NCORES)),
                             trace=TRACE)
    if TRACE:
        _CACHE["last_result"] = r
    per_seq = np.concatenate([m["outv"].reshape(BL) for m in r.results])
    return np.float32(per_seq.mean(dtype=np.float64))


# revision 40
# speedup vs baseline: 1.0706x; 1.0255x over previous
"""BERT_BiLSTM_CRF loss (CRF NLL) Trainium2 kernel — TensorE-forward version.

Self-contained: kernel(**inputs) takes FULL inputs, shards batch across 8
NeuronCores (128 seqs/core), returns the scalar mean loss.

Forward: the CRF recurrence S_t = diag(ef_t) E S_{t-1} (exp domain,
G-centered emissions) runs on the TensorEngine in a transposed layout:
partitions = 16 groups x 8 states (7 real + 1 dummy), columns = sets x 128
seqs; blk = set*16+g gives NBLK=128 time blocks of C=16. Per step and
column-half: one block-diagonal [128x128]x[128,512] matmul + one DVE mult
by transposed emissions EFT (built by DMA-XBAR transposes of exp(feats-G)).
The stationary's dummy columns hold E8 = exp(trans[STOP,:]) and the dummy
emission is 1.0, so each matmul's dummy output rows carry the previous
step's final-score dot d = E8 . S — written per step to DRAM (16 rows via
partition-strided DMA). Probe pass (last 4 taus, uniform start) seeds all
blocks (unnormalized); per-block delta = ln(end mass) - ln(probe mass)
telescopes into the per-seq offset via triangular 16x16 matmul prefix.
fwd = ln(d*) + Lam* + G*len, d*/Lam* fetched by indirect DMA at len-1.
Validated vs the exact reference in numpy (bf16 chain: loss rel 8e-7).

Gold scores: sum_t feat[t,tag_t] via eq-mask/mult/reduce on GPSIMD;
sum_t trans[tag_t,tag_{t-1}] via exact degree-6 Horner polynomials
(host-solved Vandermonde coefficients; two degree-3 custom DVE ops per row,
quadratic coeffs inlined as immediates) + eq-masked accumulation per row.
"""

import numpy as np

B, T, K = 1024, 2048, 9
NCORES = 8
BL = B // NCORES          # sequences per core (=128 partitions)
KA = 7                    # active states
JP = 8                    # padded state dim (j=7 is the d-carrier dummy)
START, STOP = 7, 8
G = 2.4                   # per-step log growth centering
C = 16                    # block length
NBLK = T // C             # 128 blocks
NSET = NBLK // 16         # 8 sets; blk = set*16 + g
P = 128                   # partitions in transposed layout: p = g*8 + j
COLS = NSET * BL          # 1024 columns: n = set*128 + b
HC = COLS // 2            # column half (4 sets)
FCH = T // 16             # feats DMA chunk (128 timesteps = 8 blocks)

_CACHE = {}
TRACE = False


def _register_dve_ops():
    import concourse.dve_ops as DO
    from concourse.dve_spec import Spec, Src0, Src1, C0, C1, C2, C3, eq, \
        lower, _spill_c3_to_src1
    from concourse.dve_uop import DveOpSpec
    from concourse.dve_spec import AluOp as SAluOp

    existing = {o.name: o for o in DO.OPS}

    def mk(name, spec, subdim=False):
        if name in existing:
            return existing[name]
        op = DO.DveOp(name, spec, subdim, uops_sha={})
        DO.OPS.append(op)
        DO.CUSTOM_DVE_SPECS[name] = spec
        DO._SUB_OPCODE_FOR_NAME[name] = DO._CUSTOM_DVE_ROW_BASE + len(DO.OPS) - 1
        for ver in ("v3", "v4"):
            r = DveOpSpec(name=name, opcode=DO.get_dve_sub_opcode(name),
                          uops=lower(spec, ver=ver), rd1_en=DO.has_src1(spec))
            op.uops_sha[ver] = r.sha(ver)
        return op

    def _eqmul_ref(in0, in1, s0, s1, imm2):
        out = (np.asarray(in0, np.float32) == s0) * np.asarray(in1, np.float32)
        acc = (s1 if isinstance(s1, float) else np.asarray(s1, np.float32)) \
            + out.sum(axis=1, keepdims=True)
        return out, acc

    def _h3top_ref(in0, in1, s0, s1, imm2):
        x = np.asarray(in0, np.float32)
        c3 = np.asarray(in1, np.float32)
        return s0 + x * (s1 + x * (imm2 + x * c3))

    def _h3step_ref(in0, in1, s0, s1, imm2):
        x = np.asarray(in0, np.float32)
        h = np.asarray(in1, np.float32)
        return s0 + x * (s1 + x * (imm2 + x * h))

    ops = {}
    # accum_out = s1 + sum_n (Src0==s0)*Src1
    ops["ANT_EQMUL_ACC"] = mk(
        "ANT_EQMUL_ACC",
        Spec(body=eq(Src0, C0) * Src1, accum=SAluOp.ADD, accum_init=C1,
             reference=_eqmul_ref))
    # out = s0 + x*(s1 + x*(imm2 + x*c3)) with c3 spilled to in1 (elem 0)
    ops["ANT_H3_TOP"] = mk(
        "ANT_H3_TOP",
        Spec(body=_spill_c3_to_src1(C0 + Src0 * (C1 + Src0 * (C2 + Src0 * C3))),
             reference=_h3top_ref))
    # out = s0 + x*(s1 + x*(imm2 + x*Src1))
    ops["ANT_H3_STEP"] = mk(
        "ANT_H3_STEP",
        Spec(body=C0 + Src0 * (C1 + Src0 * (C2 + Src0 * Src1)),
             reference=_h3step_ref))
    return ops


def _build_bass(coefs_host):
    import concourse.bass as bass
    import concourse.bacc as bacc
    import concourse.tile as tile
    import concourse.mybir as mybir

    OPS = _register_dve_ops()

    f32 = mybir.dt.float32
    bf16 = mybir.dt.bfloat16
    i32 = mybir.dt.int32
    AX = mybir.AxisListType
    OP = mybir.AluOpType
    AF = mybir.ActivationFunctionType

    nc = bacc.Bacc()

    feats = nc.dram_tensor("feats", [BL, T, K], f32, kind="ExternalInput")
    tagf = nc.dram_tensor("tagf", [BL, T], f32, kind="ExternalInput")
    lenf = nc.dram_tensor("lenf", [BL, 1], f32, kind="ExternalInput")
    leni = nc.dram_tensor("leni", [BL, 1], i32, kind="ExternalInput")
    trans = nc.dram_tensor("trans", [K, K], f32, kind="ExternalInput")
    # Horner coefficients (host Vandermonde): rows 0..6 = trans[j, x-1],
    # row 7 = trans[STOP, x-1], row 8 = trans[x-1, START]; all in x=tag+1.
    coefs = nc.dram_tensor("coefs", [9, 7], f32, kind="ExternalInput")
    outv = nc.dram_tensor("outv", [BL, 1], f32, kind="ExternalOutput")

    hist_d = nc.dram_tensor("hist_d", [(C + 1) * 16 * COLS, 1], bf16)
    offs_d = nc.dram_tensor("offs_d", [16 * COLS, 1], f32)

    iota_t_np = np.arange(T, dtype=np.float32).reshape(1, T)
    c_iota_t = nc.inline_tensor(iota_t_np, "c_iota_t")
    c_b32 = nc.inline_tensor(np.arange(BL, dtype=np.int32).reshape(BL, 1),
                             "c_b32")
    c_sTm1 = nc.inline_tensor((np.arange(BL, dtype=np.int64) * T - 1)
                              .astype(np.int32).reshape(BL, 1), "c_sTm1")
    onebd_np = np.zeros((P, 16), np.float32)
    for g in range(16):
        onebd_np[8 * g:8 * g + 7, g] = 1.0   # exclude dummy row j=7
    c_onebd = nc.inline_tensor(onebd_np, "c_onebd")
    tri16_np = np.triu(np.ones((16, 16), np.float32), 1)  # [k,m]=1 iff k<m
    c_tri16 = nc.inline_tensor(tri16_np, "c_tri16")
    c_ones16 = nc.inline_tensor(np.ones((16, 16), np.float32), "c_ones16")

    ch = coefs_host  # [9, 7] float, for inline immediates (quadratic coefs)

    with tile.TileContext(nc) as tc:
        import contextlib
        ctx = contextlib.ExitStack()
        with ctx:
            sing = ctx.enter_context(tc.tile_pool(name="sing", bufs=1))
            epool = ctx.enter_context(tc.tile_pool(name="epool", bufs=5))
            cpool = ctx.enter_context(tc.tile_pool(name="cpool", bufs=3))
            gpool = ctx.enter_context(tc.tile_pool(name="gpool", bufs=1))
            gq = ctx.enter_context(tc.tile_pool(name="gq", bufs=1))
            spool = ctx.enter_context(tc.tile_pool(name="spool", bufs=4))
            mmps = ctx.enter_context(
                tc.tile_pool(name="mmps", bufs=2, space="PSUM"))
            upps = ctx.enter_context(
                tc.tile_pool(name="upps", bufs=1, space="PSUM"))

            # ---------- tiny constants ----------
            negG = sing.tile([BL, 1], f32)
            nc.gpsimd.memset(negG[:], -G)
            coefb = sing.tile([BL, 63], f32)
            nc.gpsimd.dma_start(coefb[:], bass.AP(coefs, 0, [[0, BL], [1, 63]]))
            iota_t = sing.tile([BL, T], f32)
            nc.gpsimd.dma_start(iota_t[:], bass.AP(c_iota_t, 0, [[0, BL], [1, T]]))
            b32 = sing.tile([BL, 1], i32)
            nc.gpsimd.dma_start(b32[:], c_b32[:, :])
            sTm1 = sing.tile([BL, 1], i32)
            nc.gpsimd.dma_start(sTm1[:], c_sTm1[:, :])
            lenf_sb = sing.tile([BL, 1], f32)
            nc.gpsimd.dma_start(lenf_sb[:], lenf[:, :])
            leni_sb = sing.tile([BL, 1], i32)
            nc.gpsimd.dma_start(leni_sb[:], leni[:, :])

            # tags: needed early for gold
            tagf_sb = sing.tile([BL, T], f32)
            nc.sync.dma_start(tagf_sb[:], tagf[:, :])

            # E^T extended tile: EtA[i, j<7] = exp(trans[j, i]),
            # EtA[i, 7] = exp(trans[STOP, i])  (final-score column)
            t7x = sing.tile([7, 8], f32)
            nc.gpsimd.dma_start(t7x[:, 0:7], bass.AP(trans, 0, [[1, 7], [9, 7]]))
            nc.gpsimd.dma_start(t7x[:, 7:8],
                                bass.AP(trans, STOP * K, [[1, 7], [1, 1]]))
            EtA = sing.tile([7, 8], bf16)
            nc.scalar.activation(EtA[:], t7x[:], AF.Exp)
            # E7[j] = exp(trans[j, START]) on partitions j
            t7b = sing.tile([7, 1], f32)
            nc.gpsimd.dma_start(t7b[:], bass.AP(trans, START, [[9, 7], [1, 1]]))
            E7e = sing.tile([7, 1], f32)
            nc.scalar.activation(E7e[:], t7b[:], AF.Exp)
            # rsE[j] = sum_i E[j,i] (no matmul: row-major E tile + reduce)
            t7r = sing.tile([7, 7], f32)
            nc.gpsimd.dma_start(t7r[:], bass.AP(trans, 0, [[9, 7], [1, 7]]))
            Ete = sing.tile([7, 7], f32)
            nc.scalar.activation(Ete[:], t7r[:], AF.Exp)
            rs7 = sing.tile([7, 1], f32)
            nc.vector.tensor_reduce(out=rs7[:], in_=Ete[:], axis=AX.X, op=OP.add)
            rsE = sing.tile([P, 1], f32)
            nc.gpsimd.memset(rsE[:], 0.0)
            for g in range(16):
                nc.gpsimd.dma_start(rsE[8 * g:8 * g + 7, :], rs7[:, :])

            # ---------- stationary matrices ----------
            # EB blockdiag: col (g,j<7) = E^T block; col (g,7) = E8 (so each
            # matmul's dummy output rows carry d = E8 . S of the prev step)
            EB = sing.tile([P, P], bf16)
            nc.vector.memset(EB[:], 0.0)
            for g in range(16):
                nc.gpsimd.dma_start(EB[8 * g:8 * g + 7, 8 * g:8 * g + 8],
                                    EtA[:, :])
            ONEBD = sing.tile([P, 16], bf16)      # [k=(g,i<7), m=g'] = 1[g=g']
            nc.gpsimd.dma_start(ONEBD[:], c_onebd[:, :])
            TRI16 = sing.tile([16, 16], f32)      # [k, m] = 1[k < m]
            nc.gpsimd.dma_start(TRI16[:], c_tri16[:, :])
            ONES16 = sing.tile([16, 16], f32)
            nc.gpsimd.dma_start(ONES16[:], c_ones16[:, :])

            # ---------- feats DMA + emissions + XBAR transposes ----------
            featsb = sing.tile([BL, T, K], f32)
            # EFT[p=(g,j), tau, set, b] = exp(feats[b, (set*16+g)*C+tau, j]-G)
            EFT = sing.tile([P, C, NSET, BL], bf16)
            # all chunk DMAs first (no interleaved waits -> queues stream),
            # round-robin over all three DMA queues (sync/scalar HWDGE +
            # gpsimd SWDGE)
            for cidx in range(16):
                t0 = cidx * FCH
                eng = (nc.sync, nc.scalar, nc.gpsimd)[cidx % 3]
                eng.dma_start(featsb[:, t0:t0 + FCH, :],
                              feats[:, t0:t0 + FCH, :])
            for s in range(NSET):
                ef2 = epool.tile([BL, C, 16, JP], bf16, tag="ef2")
                # dummy-state emission = 1.0 so matmul d-rows ride unscaled
                nc.gpsimd.memset(ef2[:, :, :, 7:8], 1.0)
                for q in range(2):
                    cidx = s * 2 + q
                    t0 = cidx * FCH
                    inap = featsb[:, t0:t0 + FCH, 0:KA] \
                        .rearrange("p (g tau) j -> p tau g j", tau=C)
                    nc.scalar.activation(ef2[:, :, 8 * q:8 * q + 8, 0:KA], inap,
                                         AF.Exp, bias=negG[:, 0:1])
                eng = nc.sync if (s % 2 == 0) else nc.scalar
                eng.dma_start(
                    EFT[:, :, s, :],
                    ef2[:].rearrange("p tau g j -> p (tau g j)"),
                    transpose=True)

            # ---------- gold: masks ----------
            maskb = sing.tile([BL, T], bf16)
            nc.vector.tensor_tensor(maskb[:], iota_t[:],
                                    lenf_sb[:].broadcast_to([BL, T]), op=OP.is_lt)
            tagp1m = sing.tile([BL, T], bf16)
            nc.vector.scalar_tensor_tensor(tagp1m[:], tagf_sb[:], 1.0, maskb[:],
                                           op0=OP.add, op1=OP.mult)

            # ---------- gather indices ----------
            # li0 = len-1; blk = li0>>4; tau = li0&15; s = blk>>4; g = blk&15
            # idxA = (tau+1)*16384 + g*1024 + s*128 + b
            # idxB = g*1024 + s*128 + b
            def gp_const(v):
                tl = spool.tile([BL, 1], i32, tag="ic")
                nc.gpsimd.memset(tl[:], v)
                return tl

            cm1 = gp_const(-1)
            c4i = gp_const(4)
            li0 = spool.tile([BL, 1], i32, tag="li0")
            nc.vector.tensor_tensor(li0[:], leni_sb[:], cm1[:], op=OP.add)
            blkt = spool.tile([BL, 1], i32, tag="blkt")
            nc.vector.tensor_tensor(blkt[:], li0[:], c4i[:],
                                    op=OP.logical_shift_right)
            tmp = spool.tile([BL, 1], i32, tag="tmp")
            nc.vector.tensor_tensor(tmp[:], blkt[:], c4i[:],
                                    op=OP.logical_shift_left)
            taut = spool.tile([BL, 1], i32, tag="taut")
            nc.vector.tensor_tensor(taut[:], li0[:], tmp[:], op=OP.subtract)
            st = spool.tile([BL, 1], i32, tag="st")
            nc.vector.tensor_tensor(st[:], blkt[:], c4i[:],
                                    op=OP.logical_shift_right)
            tmp2 = spool.tile([BL, 1], i32, tag="tmp2")
            nc.vector.tensor_tensor(tmp2[:], st[:], c4i[:],
                                    op=OP.logical_shift_left)
            gt = spool.tile([BL, 1], i32, tag="gt")
            nc.vector.tensor_tensor(gt[:], blkt[:], tmp2[:], op=OP.subtract)

            c14i = gp_const(14)
            c10i = gp_const(10)
            c7i = gp_const(7)
            idxB = spool.tile([BL, 1], i32, tag="idxB")
            nc.vector.tensor_tensor(idxB[:], gt[:], c10i[:],
                                    op=OP.logical_shift_left)
            ts = spool.tile([BL, 1], i32, tag="ts")
            nc.vector.tensor_tensor(ts[:], st[:], c7i[:],
                                    op=OP.logical_shift_left)
            nc.vector.tensor_tensor(idxB[:], idxB[:], ts[:], op=OP.add)
            nc.vector.tensor_tensor(idxB[:], idxB[:], b32[:], op=OP.add)
            c16k = gp_const(16 * COLS)
            idxA = spool.tile([BL, 1], i32, tag="idxA")
            nc.vector.tensor_tensor(idxA[:], taut[:], c14i[:],
                                    op=OP.logical_shift_left)
            nc.vector.tensor_tensor(idxA[:], idxA[:], idxB[:], op=OP.add)
            nc.vector.tensor_tensor(idxA[:], idxA[:], c16k[:], op=OP.add)


            # ---------- gold: trans part (2x deg-3 Horner + eqmul per row) ----
            acc = sing.tile([BL, 1], f32)
            nc.vector.memset(acc[:], 0.0)
            junk = sing.tile([BL, T], f32)
            prevs = tagp1m[:, 0:T - 1]
            curs = tagp1m[:, 1:T]
            for j in range(KA):
                cj = coefb[:, j * 7: j * 7 + 7]
                h1 = gpool.tile([BL, T], f32, tag="h1")
                nc.vector._custom_dve(OPS["ANT_H3_TOP"], out=h1[:, 0:T - 1],
                                      in0=prevs, in1=cj[:, 6:7],
                                      s0=cj[:, 3:4], s1=cj[:, 4:5],
                                      imm2=float(ch[j, 5]))
                h2 = gpool.tile([BL, T], f32, tag="h2")
                nc.vector._custom_dve(OPS["ANT_H3_STEP"], out=h2[:, 0:T - 1],
                                      in0=prevs, in1=h1[:, 0:T - 1],
                                      s0=cj[:, 0:1], s1=cj[:, 1:2],
                                      imm2=float(ch[j, 2]))
                acc2 = spool.tile([BL, 1], f32, tag="acc")
                nc.vector._custom_dve(OPS["ANT_EQMUL_ACC"], out=junk[:, 0:T - 1],
                                      in0=curs, in1=h2[:, 0:T - 1],
                                      s0=float(j + 1), s1=acc[:],
                                      accum_out=acc2[:])
                acc = acc2

            # ---------- forward: probe (taus C-4..C-1), unnormalized --------
            def eft_h(tau, h):
                return EFT[:, tau, 4 * h:4 * h + 4, :] \
                    .rearrange("p s b -> p (s b)")

            yp = [None, None]
            for h in range(2):
                y0 = cpool.tile([P, HC], bf16, tag=f"y{h}")
                nc.vector.tensor_tensor(
                    y0[:], eft_h(C - 4, h),
                    rsE[:, 0:1].broadcast_to([P, HC]), op=OP.mult)
                yp[h] = y0
            for tau in range(C - 3, C):
                for h in range(2):
                    mm = mmps.tile([P, HC], f32, tag=f"mm{h}")
                    nc.tensor.matmul(mm[:], EB[:], yp[h][:])
                    y_new = cpool.tile([P, HC], bf16, tag=f"y{h}")
                    nc.vector.tensor_tensor(y_new[:], mm[:], eft_h(tau, h),
                                            op=OP.mult)
                    yp[h] = y_new

            # probe block masses: lnm0[g, n] = ln(sum_{j<7} y[(g,j), n])
            lnm0 = sing.tile([16, COLS], f32)
            for h in range(2):
                mps = upps.tile([16, HC], f32, tag="m16")
                nc.tensor.matmul(mps[:], ONEBD[:], yp[h][:])
                nc.scalar.activation(lnm0[:, h * HC:(h + 1) * HC], mps[:],
                                     AF.Ln)

            # ---------- seeds (partition-shifted: DMA, engines can't) -------
            yin = sing.tile([P, COLS], bf16)
            nc.vector.memset(yin[0:8, 0:BL], 0.0)
            for h in range(2):
                nc.sync.dma_start(yin[8:P, h * HC:(h + 1) * HC],
                                  yp[h][0:P - 8, :])
            for s in range(1, NSET):
                sh, sc = (s - 1) // 4, ((s - 1) % 4) * BL
                nc.scalar.dma_start(yin[0:8, s * BL:(s + 1) * BL],
                                    yp[sh][P - 8:P, sc:sc + BL])

            # ---------- pass 2: two independent column-half chains ----------
            # y_new = (E y)*ef; EB dummy cols carry d = E8.S of the previous
            # step into rows (g,7) (ef pad = 1.0): hist region tau holds
            # d_{tau-1}; ghost step adds d_{C-1} at region C.
            y_last = [None, None]
            for tau in range(C):
                for h in range(2):
                    y_prev = yin[:, h * HC:(h + 1) * HC] if tau == 0 \
                        else y_last[h][:]
                    mm = mmps.tile([P, HC], f32, tag=f"mm{h}")
                    nc.tensor.matmul(mm[:], EB[:], y_prev)
                    y_new = cpool.tile([P, HC], bf16, tag=f"y{h}")
                    nc.vector.tensor_tensor(y_new[:], mm[:], eft_h(tau, h),
                                            op=OP.mult)
                    if tau == 0 and h == 0:
                        # block 0 exact init: S_0 = exp(trans[j,START])*ef_0
                        nc.vector.tensor_tensor(
                            y_new[0:KA, 0:BL],
                            E7e[:, 0:1].broadcast_to([KA, BL]),
                            EFT[0:KA, 0, 0, :], op=OP.mult)
                    drows = y_new[:].rearrange("(g j) n -> g j n", j=8)[:, 7, :]
                    eng = nc.sync if h == 0 else nc.scalar
                    eng.dma_start(
                        bass.AP(hist_d, tau * 16 * COLS + h * HC,
                                [[COLS, 16], [1, HC]]), drows)
                    y_last[h] = y_new
            # ghost step: d_{C-1} shows up in MM(C)'s dummy rows
            for h in range(2):
                mm = mmps.tile([P, HC], f32, tag=f"mm{h}")
                nc.tensor.matmul(mm[:], EB[:], y_last[h][:])
                yg = cpool.tile([P, HC], bf16, tag=f"y{h}")
                nc.scalar.copy(yg[:], mm[:])
                eng = nc.sync if h == 0 else nc.scalar
                gr = yg[:].rearrange("(g j) n -> g j n", j=8)[:, 7, :]
                eng.dma_start(
                    bass.AP(hist_d, C * 16 * COLS + h * HC,
                            [[COLS, 16], [1, HC]]), gr)

            # ---------- delta + exclusive prefix (Lam) ----------
            lam_sb = sing.tile([16, COLS], f32)
            for h in range(2):
                mEp = upps.tile([16, HC], f32, tag="m16")
                nc.tensor.matmul(mEp[:], ONEBD[:], y_last[h][:])
                nc.scalar.activation(lam_sb[:, h * HC:(h + 1) * HC], mEp[:],
                                     AF.Ln)
            dlt = sing.tile([16, COLS], f32)
            nc.vector.tensor_tensor(dlt[:], lam_sb[:], lnm0[:], op=OP.subtract)
            Lam_sb = sing.tile([16, COLS], f32)
            pairs = [(sp, s) for s in range(1, NSET) for sp in range(s)]
            for hh in range(2):
                Lps = upps.tile([16, HC], f32, tag=f"lps{hh}")
                hsl = slice(hh * HC, (hh + 1) * HC)
                hp = [(sp, s) for (sp, s) in pairs if s // 4 == hh]
                nc.tensor.matmul(Lps[:], TRI16[:], dlt[:, hsl],
                                 start=True, stop=(not hp),
                                 skip_group_check=True)
                for idx, (sp, s) in enumerate(hp):
                    c0 = (s % 4) * BL
                    nc.tensor.matmul(Lps[:, c0:c0 + BL], ONES16[:],
                                     dlt[:, sp * BL:(sp + 1) * BL],
                                     start=False, stop=(idx == len(hp) - 1),
                                     skip_group_check=True)
                nc.scalar.copy(Lam_sb[:, hsl], Lps[:])
            nc.sync.dma_start(bass.AP(offs_d, 0, [[COLS, 16], [1, COLS]]),
                              Lam_sb[:])

            # ---------- gold: feats part (t-halves so half 0 starts early) ---
            facc = sing.tile([BL, 1], f32)
            nc.vector.memset(facc[:], 0.0)
            fa = facc
            TH = T // 2
            for th in range(2):
                tsl = slice(th * TH, (th + 1) * TH)
                for j in range(KA):
                    fa2 = spool.tile([BL, 1], f32, tag="fa")
                    nc.vector._custom_dve(OPS["ANT_EQMUL_ACC"],
                                          out=junk[:, 0:TH],
                                          in0=tagp1m[:, tsl],
                                          in1=featsb[:, tsl, :][:, :, j],
                                          s0=float(j + 1), s1=fa[:],
                                          accum_out=fa2[:])
                    fa = fa2
            facc = fa

            # ---------- gold: t0 term trans[tag_0, START] (coef row 8) -------
            x0 = tagp1m[:, 0:1]
            c8 = coefb[:, 56:63]
            p1 = spool.tile([BL, 1], f32, tag="p1")
            nc.vector._custom_dve(OPS["ANT_H3_TOP"], out=p1[:], in0=x0,
                                  in1=c8[:, 6:7], s0=c8[:, 3:4], s1=c8[:, 4:5],
                                  imm2=float(ch[8, 5]))
            t0p = spool.tile([BL, 1], f32, tag="t0p")
            nc.vector._custom_dve(OPS["ANT_H3_STEP"], out=t0p[:], in0=x0,
                                  in1=p1[:], s0=c8[:, 0:1], s1=c8[:, 1:2],
                                  imm2=float(ch[8, 2]))

            # ---------- gold: last term trans[STOP, tag_last] ----------
            idxT = spool.tile([BL, 1], i32, tag="idxT")
            nc.gpsimd.tensor_tensor(idxT[:], sTm1[:], leni_sb[:], op=OP.add)
            tgl = spool.tile([BL, 1], f32, tag="tgl")
            nc.gpsimd.indirect_dma_start(
                out=tgl[:], out_offset=None,
                in_=bass.AP(tagf, 0, [[1, BL * T], [1, 1]]),
                in_offset=bass.IndirectOffsetOnAxis(ap=idxT[:, 0:1], axis=0))
            f1 = spool.tile([BL, 1], f32, tag="f1")
            nc.vector.memset(f1[:], 1.0)
            xl = spool.tile([BL, 1], f32, tag="xl")
            nc.vector.tensor_tensor(xl[:], tgl[:], f1[:], op=OP.add)
            c7r = coefb[:, 49:56]
            q1 = spool.tile([BL, 1], f32, tag="q1")
            nc.vector._custom_dve(OPS["ANT_H3_TOP"], out=q1[:], in0=xl[:],
                                  in1=c7r[:, 6:7], s0=c7r[:, 3:4],
                                  s1=c7r[:, 4:5], imm2=float(ch[7, 5]))
            lastp = spool.tile([BL, 1], f32, tag="lastp")
            nc.vector._custom_dve(OPS["ANT_H3_STEP"], out=lastp[:], in0=xl[:],
                                  in1=q1[:], s0=c7r[:, 0:1], s1=c7r[:, 1:2],
                                  imm2=float(ch[7, 2]))

            # ---------- gathers ----------
            dg = spool.tile([BL, 1], bf16, tag="dg")
            nc.gpsimd.indirect_dma_start(
                out=dg[:], out_offset=None,
                in_=bass.AP(hist_d, 0, [[1, (C + 1) * 16 * COLS], [1, 1]]),
                in_offset=bass.IndirectOffsetOnAxis(ap=idxA[:, 0:1], axis=0))
            offg = spool.tile([BL, 1], f32, tag="offg")
            nc.gpsimd.indirect_dma_start(
                out=offg[:], out_offset=None,
                in_=bass.AP(offs_d, 0, [[1, 16 * COLS], [1, 1]]),
                in_offset=bass.IndirectOffsetOnAxis(ap=idxB[:, 0:1], axis=0))

            # ---------- finalize ----------
            lnv = spool.tile([BL, 1], f32, tag="lnv")
            nc.scalar.activation(lnv[:], dg[:], AF.Ln)
            fwd1 = spool.tile([BL, 1], f32, tag="fwd1")
            nc.vector.tensor_tensor(fwd1[:], lnv[:], offg[:], op=OP.add)
            fwd2 = spool.tile([BL, 1], f32, tag="fwd2")
            nc.vector.scalar_tensor_tensor(fwd2[:], lenf_sb[:], G, fwd1[:],
                                           op0=OP.mult, op1=OP.add)
            g2 = spool.tile([BL, 1], f32, tag="g2")
            nc.vector.tensor_tensor(g2[:], t0p[:], lastp[:], op=OP.add)
            g3 = spool.tile([BL, 1], f32, tag="g3")
            nc.vector.tensor_tensor(g3[:], acc[:], g2[:], op=OP.add)
            g4 = spool.tile([BL, 1], f32, tag="g4")
            nc.vector.tensor_tensor(g4[:], g3[:], facc[:], op=OP.add)
            res = spool.tile([BL, 1], f32, tag="res")
            nc.vector.tensor_tensor(res[:], fwd2[:], g4[:], op=OP.subtract)
            nc.sync.dma_start(outv[:, :], res[:])

    nc.finalize()
    return nc


def _coefs(transitions):
    tr = np.asarray(transitions, np.float64)
    V = np.vander(np.arange(1, 8, dtype=np.float64), 7, increasing=True)
    rows = [np.linalg.solve(V, tr[j, 0:7]) for j in range(7)]
    rows.append(np.linalg.solve(V, tr[STOP, 0:7]))
    rows.append(np.linalg.solve(V, tr[0:7, START]))
    return np.stack(rows).astype(np.float32)


def kernel(feats, transitions, tags, lengths):
    feats = np.ascontiguousarray(np.asarray(feats, dtype=np.float32))
    transitions = np.ascontiguousarray(np.asarray(transitions, dtype=np.float32))
    tags_f = np.ascontiguousarray(np.asarray(tags).astype(np.float32))
    len_f = np.ascontiguousarray(np.asarray(lengths).astype(np.float32).reshape(B, 1))
    len_i = np.ascontiguousarray(np.asarray(lengths).astype(np.int32).reshape(B, 1))
    coefs = np.ascontiguousarray(_coefs(transitions))

    key = ("nc", transitions.tobytes())
    if key not in _CACHE:
        _CACHE[key] = _build_bass(coefs.astype(np.float64))
    nc = _CACHE[key]

    from concourse.bass_utils import run_bass_kernel_spmd

    in_maps = []
    for c in range(NCORES):
        sl = slice(c * BL, (c + 1) * BL)
        in_maps.append({
            "feats": feats[sl],
            "tagf": tags_f[sl],
            "lenf": len_f[sl],
            "leni": len_i[sl],
            "trans": transitions,
            "coefs": coefs,
        })
    r = run_bass_kernel_spmd(nc, in_maps, core_ids=list(range(NCORES)),
                             trace=TRACE)
    if TRACE:
        _CACHE["last_result"] = r
    per_seq = np.concatenate([m["outv"].reshape(BL) for m in r.results])
    return np.float32(per_seq.mean(dtype=np.float64))


# revision 41
# speedup vs baseline: 1.1608x; 1.0843x over previous
"""BERT_BiLSTM_CRF loss (CRF NLL) Trainium2 kernel — TensorE-forward version.

Self-contained: kernel(**inputs) takes FULL inputs, shards batch across 8
NeuronCores (128 seqs/core), returns the scalar mean loss.

Forward: the CRF recurrence S_t = diag(ef_t) E S_{t-1} (exp domain,
G-centered emissions) runs on the TensorEngine in a transposed layout:
partitions = 16 groups x 8 states (7 real + 1 dummy), columns = sets x 128
seqs; blk = set*16+g gives NBLK=128 time blocks of C=16. Per step and
column-half: one block-diagonal [128x128]x[128,512] matmul + one DVE mult
by transposed emissions EFT (built by DMA-XBAR transposes of exp(feats-G)).
The stationary's dummy columns hold E8 = exp(trans[STOP,:]) and the dummy
emission is 1.0, so each matmul's dummy output rows carry the previous
step's final-score dot d = E8 . S — written per step to DRAM (16 rows via
partition-strided DMA). Probe pass (last 4 taus, uniform start) seeds all
blocks (unnormalized); per-block delta = ln(end mass) - ln(probe mass)
telescopes into the per-seq offset via triangular 16x16 matmul prefix.
fwd = ln(d*) + Lam* + G*len, d*/Lam* fetched by indirect DMA at len-1.
Validated vs the exact reference in numpy (bf16 chain: loss rel 8e-7).

Gold scores: sum_t feat[t,tag_t] via eq-mask/mult/reduce on GPSIMD;
sum_t trans[tag_t,tag_{t-1}] via exact degree-6 Horner polynomials
(host-solved Vandermonde coefficients; two degree-3 custom DVE ops per row,
quadratic coeffs inlined as immediates) + eq-masked accumulation per row.
"""

import numpy as np

B, T, K = 1024, 2048, 9
NCORES = 8
BL = B // NCORES          # sequences per core (=128 partitions)
KA = 7                    # active states
JP = 8                    # padded state dim (j=7 is the d-carrier dummy)
START, STOP = 7, 8
G = 2.4                   # per-step log growth centering
C = 16                    # block length
NBLK = T // C             # 128 blocks
NSET = NBLK // 16         # 8 sets; blk = set*16 + g
P = 128                   # partitions in transposed layout: p = g*8 + j
COLS = NSET * BL          # 1024 columns: n = set*128 + b
HC = COLS // 2            # column half (4 sets)
FCH = T // 16             # feats DMA chunk (128 timesteps = 8 blocks)

_CACHE = {}
TRACE = False


def _register_dve_ops():
    import concourse.dve_ops as DO
    from concourse.dve_spec import Spec, Src0, Src1, C0, C1, C2, C3, eq, \
        lower, _spill_c3_to_src1
    from concourse.dve_uop import DveOpSpec
    from concourse.dve_spec import AluOp as SAluOp

    existing = {o.name: o for o in DO.OPS}

    def mk(name, spec, subdim=False):
        if name in existing:
            return existing[name]
        op = DO.DveOp(name, spec, subdim, uops_sha={})
        DO.OPS.append(op)
        DO.CUSTOM_DVE_SPECS[name] = spec
        DO._SUB_OPCODE_FOR_NAME[name] = DO._CUSTOM_DVE_ROW_BASE + len(DO.OPS) - 1
        for ver in ("v3", "v4"):
            r = DveOpSpec(name=name, opcode=DO.get_dve_sub_opcode(name),
                          uops=lower(spec, ver=ver), rd1_en=DO.has_src1(spec))
            op.uops_sha[ver] = r.sha(ver)
        return op

    def _eqmul_ref(in0, in1, s0, s1, imm2):
        out = (np.asarray(in0, np.float32) == s0) * np.asarray(in1, np.float32)
        acc = (s1 if isinstance(s1, float) else np.asarray(s1, np.float32)) \
            + out.sum(axis=1, keepdims=True)
        return out, acc

    def _h3top_ref(in0, in1, s0, s1, imm2):
        x = np.asarray(in0, np.float32)
        c3 = np.asarray(in1, np.float32)
        return s0 + x * (s1 + x * (imm2 + x * c3))

    def _h3step_ref(in0, in1, s0, s1, imm2):
        x = np.asarray(in0, np.float32)
        h = np.asarray(in1, np.float32)
        return s0 + x * (s1 + x * (imm2 + x * h))

    ops = {}
    # accum_out = s1 + sum_n (Src0==s0)*Src1
    ops["ANT_EQMUL_ACC"] = mk(
        "ANT_EQMUL_ACC",
        Spec(body=eq(Src0, C0) * Src1, accum=SAluOp.ADD, accum_init=C1,
             reference=_eqmul_ref))
    # out = s0 + x*(s1 + x*(imm2 + x*c3)) with c3 spilled to in1 (elem 0)
    ops["ANT_H3_TOP"] = mk(
        "ANT_H3_TOP",
        Spec(body=_spill_c3_to_src1(C0 + Src0 * (C1 + Src0 * (C2 + Src0 * C3))),
             reference=_h3top_ref))
    # out = s0 + x*(s1 + x*(imm2 + x*Src1))
    ops["ANT_H3_STEP"] = mk(
        "ANT_H3_STEP",
        Spec(body=C0 + Src0 * (C1 + Src0 * (C2 + Src0 * Src1)),
             reference=_h3step_ref))
    return ops


def _build_bass(coefs_host):
    import concourse.bass as bass
    import concourse.bacc as bacc
    import concourse.tile as tile
    import concourse.mybir as mybir

    OPS = _register_dve_ops()

    f32 = mybir.dt.float32
    bf16 = mybir.dt.bfloat16
    i32 = mybir.dt.int32
    AX = mybir.AxisListType
    OP = mybir.AluOpType
    AF = mybir.ActivationFunctionType

    nc = bacc.Bacc()

    feats = nc.dram_tensor("feats", [BL, T, K], f32, kind="ExternalInput")
    tagf = nc.dram_tensor("tagf", [BL, T], f32, kind="ExternalInput")
    lenf = nc.dram_tensor("lenf", [BL, 1], f32, kind="ExternalInput")
    leni = nc.dram_tensor("leni", [BL, 1], i32, kind="ExternalInput")
    trans = nc.dram_tensor("trans", [K, K], f32, kind="ExternalInput")
    # Horner coefficients (host Vandermonde): rows 0..6 = trans[j, x-1],
    # row 7 = trans[STOP, x-1], row 8 = trans[x-1, START]; all in x=tag+1.
    coefs = nc.dram_tensor("coefs", [9, 7], f32, kind="ExternalInput")
    outv = nc.dram_tensor("outv", [BL, 1], f32, kind="ExternalOutput")

    hist_d = nc.dram_tensor("hist_d", [(C + 1) * 16 * COLS, 1], bf16)
    offs_d = nc.dram_tensor("offs_d", [16 * COLS, 1], f32)

    iota_t_np = np.arange(T, dtype=np.float32).reshape(1, T)
    c_iota_t = nc.inline_tensor(iota_t_np, "c_iota_t")
    c_b32 = nc.inline_tensor(np.arange(BL, dtype=np.int32).reshape(BL, 1),
                             "c_b32")
    c_sTm1 = nc.inline_tensor((np.arange(BL, dtype=np.int64) * T - 1)
                              .astype(np.int32).reshape(BL, 1), "c_sTm1")
    onebd_np = np.zeros((P, 16), np.float32)
    for g in range(16):
        onebd_np[8 * g:8 * g + 7, g] = 1.0   # exclude dummy row j=7
    c_onebd = nc.inline_tensor(onebd_np, "c_onebd")
    tri16_np = np.triu(np.ones((16, 16), np.float32), 1)  # [k,m]=1 iff k<m
    c_tri16 = nc.inline_tensor(tri16_np, "c_tri16")
    c_ones16 = nc.inline_tensor(np.ones((16, 16), np.float32), "c_ones16")

    ch = coefs_host  # [9, 7] float, for inline immediates (quadratic coefs)

    with tile.TileContext(nc) as tc:
        import contextlib
        ctx = contextlib.ExitStack()
        with ctx:
            sing = ctx.enter_context(tc.tile_pool(name="sing", bufs=1))
            epool = ctx.enter_context(tc.tile_pool(name="epool", bufs=5))
            cpool = ctx.enter_context(tc.tile_pool(name="cpool", bufs=3))
            gpool = ctx.enter_context(tc.tile_pool(name="gpool", bufs=1))
            gq = ctx.enter_context(tc.tile_pool(name="gq", bufs=1))
            spool = ctx.enter_context(tc.tile_pool(name="spool", bufs=4))
            mmps = ctx.enter_context(
                tc.tile_pool(name="mmps", bufs=2, space="PSUM"))
            upps = ctx.enter_context(
                tc.tile_pool(name="upps", bufs=1, space="PSUM"))

            # ---------- tiny constants ----------
            negG = sing.tile([BL, 1], f32)
            nc.gpsimd.memset(negG[:], -G)
            coefb = sing.tile([BL, 63], f32)
            nc.gpsimd.dma_start(coefb[:], bass.AP(coefs, 0, [[0, BL], [1, 63]]))
            iota_t = sing.tile([BL, T], f32)
            nc.gpsimd.dma_start(iota_t[:], bass.AP(c_iota_t, 0, [[0, BL], [1, T]]))
            b32 = sing.tile([BL, 1], i32)
            nc.gpsimd.dma_start(b32[:], c_b32[:, :])
            sTm1 = sing.tile([BL, 1], i32)
            nc.gpsimd.dma_start(sTm1[:], c_sTm1[:, :])
            lenf_sb = sing.tile([BL, 1], f32)
            nc.gpsimd.dma_start(lenf_sb[:], lenf[:, :])
            leni_sb = sing.tile([BL, 1], i32)
            nc.gpsimd.dma_start(leni_sb[:], leni[:, :])

            # tags: needed early for gold
            tagf_sb = sing.tile([BL, T], f32)
            nc.scalar.dma_start(tagf_sb[:], tagf[:, :])

            # E^T extended tile: EtA[i, j<7] = exp(trans[j, i]),
            # EtA[i, 7] = exp(trans[STOP, i])  (final-score column)
            t7x = sing.tile([7, 8], f32)
            nc.gpsimd.dma_start(t7x[:, 0:7], bass.AP(trans, 0, [[1, 7], [9, 7]]))
            nc.gpsimd.dma_start(t7x[:, 7:8],
                                bass.AP(trans, STOP * K, [[1, 7], [1, 1]]))
            EtA = sing.tile([7, 8], bf16)
            nc.scalar.activation(EtA[:], t7x[:], AF.Exp)
            # E7[j] = exp(trans[j, START]) on partitions j
            t7b = sing.tile([7, 1], f32)
            nc.gpsimd.dma_start(t7b[:], bass.AP(trans, START, [[9, 7], [1, 1]]))
            E7e = sing.tile([7, 1], f32)
            nc.scalar.activation(E7e[:], t7b[:], AF.Exp)
            # rsE[j] = sum_i E[j,i] (no matmul: row-major E tile + reduce)
            t7r = sing.tile([7, 7], f32)
            nc.gpsimd.dma_start(t7r[:], bass.AP(trans, 0, [[9, 7], [1, 7]]))
            Ete = sing.tile([7, 7], f32)
            nc.scalar.activation(Ete[:], t7r[:], AF.Exp)
            rs7 = sing.tile([7, 1], f32)
            nc.vector.tensor_reduce(out=rs7[:], in_=Ete[:], axis=AX.X, op=OP.add)
            rsE = sing.tile([P, 1], f32)
            nc.gpsimd.memset(rsE[:], 0.0)
            for g in range(16):
                nc.gpsimd.dma_start(rsE[8 * g:8 * g + 7, :], rs7[:, :])

            # ---------- stationary matrices ----------
            # EB blockdiag: col (g,j<7) = E^T block; col (g,7) = E8 (so each
            # matmul's dummy output rows carry d = E8 . S of the prev step)
            EB = sing.tile([P, P], bf16)
            nc.vector.memset(EB[:], 0.0)
            for g in range(16):
                nc.gpsimd.dma_start(EB[8 * g:8 * g + 7, 8 * g:8 * g + 8],
                                    EtA[:, :])
            ONEBD = sing.tile([P, 16], bf16)      # [k=(g,i<7), m=g'] = 1[g=g']
            nc.gpsimd.dma_start(ONEBD[:], c_onebd[:, :])
            TRI16 = sing.tile([16, 16], f32)      # [k, m] = 1[k < m]
            nc.gpsimd.dma_start(TRI16[:], c_tri16[:, :])
            ONES16 = sing.tile([16, 16], f32)
            nc.gpsimd.dma_start(ONES16[:], c_ones16[:, :])

            # ---------- feats DMA + emissions + XBAR transposes ----------
            featsb = sing.tile([BL, T, K], f32)
            # EFT[p=(g,j), tau, set, b] = exp(feats[b, (set*16+g)*C+tau, j]-G)
            EFT = sing.tile([P, C, NSET, BL], bf16)
            # all chunk DMAs first (no interleaved waits -> queues stream),
            # round-robin over all three DMA queues (sync/scalar HWDGE +
            # gpsimd SWDGE)
            for cidx in range(16):
                t0 = cidx * FCH
                eng = nc.sync if (cidx % 2 == 0) else nc.gpsimd
                eng.dma_start(featsb[:, t0:t0 + FCH, :],
                              feats[:, t0:t0 + FCH, :])
            for s in range(NSET):
                ef2 = epool.tile([BL, C, 16, JP], bf16, tag="ef2")
                # dummy-state emission = 1.0 so matmul d-rows ride unscaled
                nc.gpsimd.memset(ef2[:, :, :, 7:8], 1.0)
                for q in range(2):
                    cidx = s * 2 + q
                    t0 = cidx * FCH
                    inap = featsb[:, t0:t0 + FCH, 0:KA] \
                        .rearrange("p (g tau) j -> p tau g j", tau=C)
                    nc.scalar.activation(ef2[:, :, 8 * q:8 * q + 8, 0:KA], inap,
                                         AF.Exp, bias=negG[:, 0:1])
                nc.sync.dma_start(
                    EFT[:, :, s, :],
                    ef2[:].rearrange("p tau g j -> p (tau g j)"),
                    transpose=True)

            # ---------- gold: masks ----------
            maskb = sing.tile([BL, T], bf16)
            nc.vector.tensor_tensor(maskb[:], iota_t[:],
                                    lenf_sb[:].broadcast_to([BL, T]), op=OP.is_lt)
            tagp1m = sing.tile([BL, T], bf16)
            nc.vector.scalar_tensor_tensor(tagp1m[:], tagf_sb[:], 1.0, maskb[:],
                                           op0=OP.add, op1=OP.mult)

            # ---------- gather indices ----------
            # li0 = len-1; blk = li0>>4; tau = li0&15; s = blk>>4; g = blk&15
            # idxA = (tau+1)*16384 + g*1024 + s*128 + b
            # idxB = g*1024 + s*128 + b
            def gp_const(v):
                tl = spool.tile([BL, 1], i32, tag="ic")
                nc.gpsimd.memset(tl[:], v)
                return tl

            cm1 = gp_const(-1)
            c4i = gp_const(4)
            li0 = spool.tile([BL, 1], i32, tag="li0")
            nc.vector.tensor_tensor(li0[:], leni_sb[:], cm1[:], op=OP.add)
            blkt = spool.tile([BL, 1], i32, tag="blkt")
            nc.vector.tensor_tensor(blkt[:], li0[:], c4i[:],
                                    op=OP.logical_shift_right)
            tmp = spool.tile([BL, 1], i32, tag="tmp")
            nc.vector.tensor_tensor(tmp[:], blkt[:], c4i[:],
                                    op=OP.logical_shift_left)
            taut = spool.tile([BL, 1], i32, tag="taut")
            nc.vector.tensor_tensor(taut[:], li0[:], tmp[:], op=OP.subtract)
            st = spool.tile([BL, 1], i32, tag="st")
            nc.vector.tensor_tensor(st[:], blkt[:], c4i[:],
                                    op=OP.logical_shift_right)
            tmp2 = spool.tile([BL, 1], i32, tag="tmp2")
            nc.vector.tensor_tensor(tmp2[:], st[:], c4i[:],
                                    op=OP.logical_shift_left)
            gt = spool.tile([BL, 1], i32, tag="gt")
            nc.vector.tensor_tensor(gt[:], blkt[:], tmp2[:], op=OP.subtract)

            c14i = gp_const(14)
            c10i = gp_const(10)
            c7i = gp_const(7)
            idxB = spool.tile([BL, 1], i32, tag="idxB")
            nc.vector.tensor_tensor(idxB[:], gt[:], c10i[:],
                                    op=OP.logical_shift_left)
            ts = spool.tile([BL, 1], i32, tag="ts")
            nc.vector.tensor_tensor(ts[:], st[:], c7i[:],
                                    op=OP.logical_shift_left)
            nc.vector.tensor_tensor(idxB[:], idxB[:], ts[:], op=OP.add)
            nc.vector.tensor_tensor(idxB[:], idxB[:], b32[:], op=OP.add)
            c16k = gp_const(16 * COLS)
            idxA = spool.tile([BL, 1], i32, tag="idxA")
            nc.vector.tensor_tensor(idxA[:], taut[:], c14i[:],
                                    op=OP.logical_shift_left)
            nc.vector.tensor_tensor(idxA[:], idxA[:], idxB[:], op=OP.add)
            nc.vector.tensor_tensor(idxA[:], idxA[:], c16k[:], op=OP.add)


            # ---------- gold: trans part (2x deg-3 Horner + eqmul per row) ----
            acc = sing.tile([BL, 1], f32)
            nc.vector.memset(acc[:], 0.0)
            junk = sing.tile([BL, T], f32)
            prevs = tagp1m[:, 0:T - 1]
            curs = tagp1m[:, 1:T]
            for j in range(KA):
                cj = coefb[:, j * 7: j * 7 + 7]
                h1 = gpool.tile([BL, T], f32, tag="h1")
                nc.vector._custom_dve(OPS["ANT_H3_TOP"], out=h1[:, 0:T - 1],
                                      in0=prevs, in1=cj[:, 6:7],
                                      s0=cj[:, 3:4], s1=cj[:, 4:5],
                                      imm2=float(ch[j, 5]))
                h2 = gpool.tile([BL, T], f32, tag="h2")
                nc.vector._custom_dve(OPS["ANT_H3_STEP"], out=h2[:, 0:T - 1],
                                      in0=prevs, in1=h1[:, 0:T - 1],
                                      s0=cj[:, 0:1], s1=cj[:, 1:2],
                                      imm2=float(ch[j, 2]))
                acc2 = spool.tile([BL, 1], f32, tag="acc")
                nc.vector._custom_dve(OPS["ANT_EQMUL_ACC"], out=junk[:, 0:T - 1],
                                      in0=curs, in1=h2[:, 0:T - 1],
                                      s0=float(j + 1), s1=acc[:],
                                      accum_out=acc2[:])
                acc = acc2

            # ---------- forward: probe (taus C-4..C-1), unnormalized --------
            def eft_h(tau, h):
                return EFT[:, tau, 4 * h:4 * h + 4, :] \
                    .rearrange("p s b -> p (s b)")

            yp = [None, None]
            for h in range(2):
                y0 = cpool.tile([P, HC], bf16, tag=f"y{h}")
                nc.vector.tensor_tensor(
                    y0[:], eft_h(C - 4, h),
                    rsE[:, 0:1].broadcast_to([P, HC]), op=OP.mult)
                yp[h] = y0
            for tau in range(C - 3, C):
                for h in range(2):
                    mm = mmps.tile([P, HC], f32, tag=f"mm{h}")
                    nc.tensor.matmul(mm[:], EB[:], yp[h][:])
                    y_new = cpool.tile([P, HC], bf16, tag=f"y{h}")
                    nc.vector.tensor_tensor(y_new[:], mm[:], eft_h(tau, h),
                                            op=OP.mult)
                    yp[h] = y_new

            # probe block masses: lnm0[g, n] = ln(sum_{j<7} y[(g,j), n])
            lnm0 = sing.tile([16, COLS], f32)
            for h in range(2):
                mps = upps.tile([16, HC], f32, tag="m16")
                nc.tensor.matmul(mps[:], ONEBD[:], yp[h][:])
                nc.scalar.activation(lnm0[:, h * HC:(h + 1) * HC], mps[:],
                                     AF.Ln)

            # ---------- seeds (partition-shifted: DMA, engines can't) -------
            yin = sing.tile([P, COLS], bf16)
            nc.vector.memset(yin[0:8, 0:BL], 0.0)
            for h in range(2):
                nc.sync.dma_start(yin[8:P, h * HC:(h + 1) * HC],
                                  yp[h][0:P - 8, :])
            for s in range(1, NSET):
                sh, sc = (s - 1) // 4, ((s - 1) % 4) * BL
                nc.gpsimd.dma_start(yin[0:8, s * BL:(s + 1) * BL],
                                    yp[sh][P - 8:P, sc:sc + BL])

            # ---------- pass 2: two independent column-half chains ----------
            # y_new = (E y)*ef; EB dummy cols carry d = E8.S of the previous
            # step into rows (g,7) (ef pad = 1.0): hist region tau holds
            # d_{tau-1}; ghost step adds d_{C-1} at region C.
            y_last = [None, None]
            for tau in range(C):
                for h in range(2):
                    y_prev = yin[:, h * HC:(h + 1) * HC] if tau == 0 \
                        else y_last[h][:]
                    mm = mmps.tile([P, HC], f32, tag=f"mm{h}")
                    nc.tensor.matmul(mm[:], EB[:], y_prev)
                    y_new = cpool.tile([P, HC], bf16, tag=f"y{h}")
                    nc.vector.tensor_tensor(y_new[:], mm[:], eft_h(tau, h),
                                            op=OP.mult)
                    if tau == 0 and h == 0:
                        # block 0 exact init: S_0 = exp(trans[j,START])*ef_0
                        nc.vector.tensor_tensor(
                            y_new[0:KA, 0:BL],
                            E7e[:, 0:1].broadcast_to([KA, BL]),
                            EFT[0:KA, 0, 0, :], op=OP.mult)
                    drows = y_new[:].rearrange("(g j) n -> g j n", j=8)[:, 7, :]
                    eng = nc.sync if h == 0 else nc.gpsimd
                    eng.dma_start(
                        bass.AP(hist_d, tau * 16 * COLS + h * HC,
                                [[COLS, 16], [1, HC]]), drows)
                    y_last[h] = y_new
            # ghost step: d_{C-1} shows up in MM(C)'s dummy rows
            for h in range(2):
                mm = mmps.tile([P, HC], f32, tag=f"mm{h}")
                nc.tensor.matmul(mm[:], EB[:], y_last[h][:])
                yg = cpool.tile([P, HC], bf16, tag=f"y{h}")
                nc.scalar.copy(yg[:], mm[:])
                eng = nc.sync if h == 0 else nc.gpsimd
                gr = yg[:].rearrange("(g j) n -> g j n", j=8)[:, 7, :]
                eng.dma_start(
                    bass.AP(hist_d, C * 16 * COLS + h * HC,
                            [[COLS, 16], [1, HC]]), gr)

            # ---------- delta + exclusive prefix (Lam) ----------
            lam_sb = sing.tile([16, COLS], f32)
            for h in range(2):
                mEp = upps.tile([16, HC], f32, tag="m16")
                nc.tensor.matmul(mEp[:], ONEBD[:], y_last[h][:])
                nc.scalar.activation(lam_sb[:, h * HC:(h + 1) * HC], mEp[:],
                                     AF.Ln)
            dlt = sing.tile([16, COLS], f32)
            nc.vector.tensor_tensor(dlt[:], lam_sb[:], lnm0[:], op=OP.subtract)
            Lam_sb = sing.tile([16, COLS], f32)
            pairs = [(sp, s) for s in range(1, NSET) for sp in range(s)]
            for hh in range(2):
                Lps = upps.tile([16, HC], f32, tag=f"lps{hh}")
                hsl = slice(hh * HC, (hh + 1) * HC)
                hp = [(sp, s) for (sp, s) in pairs if s // 4 == hh]
                nc.tensor.matmul(Lps[:], TRI16[:], dlt[:, hsl],
                                 start=True, stop=(not hp),
                                 skip_group_check=True)
                for idx, (sp, s) in enumerate(hp):
                    c0 = (s % 4) * BL
                    nc.tensor.matmul(Lps[:, c0:c0 + BL], ONES16[:],
                                     dlt[:, sp * BL:(sp + 1) * BL],
                                     start=False, stop=(idx == len(hp) - 1),
                                     skip_group_check=True)
                nc.scalar.copy(Lam_sb[:, hsl], Lps[:])
            nc.sync.dma_start(bass.AP(offs_d, 0, [[COLS, 16], [1, COLS]]),
                              Lam_sb[:])

            # ---------- gold: feats part (t-halves so half 0 starts early) ---
            facc = sing.tile([BL, 1], f32)
            nc.vector.memset(facc[:], 0.0)
            fa = facc
            TH = T // 2
            for th in range(2):
                tsl = slice(th * TH, (th + 1) * TH)
                for j in range(KA):
                    fa2 = spool.tile([BL, 1], f32, tag="fa")
                    nc.vector._custom_dve(OPS["ANT_EQMUL_ACC"],
                                          out=junk[:, 0:TH],
                                          in0=tagp1m[:, tsl],
                                          in1=featsb[:, tsl, :][:, :, j],
                                          s0=float(j + 1), s1=fa[:],
                                          accum_out=fa2[:])
                    fa = fa2
            facc = fa

            # ---------- gold: t0 term trans[tag_0, START] (coef row 8) -------
            x0 = tagp1m[:, 0:1]
            c8 = coefb[:, 56:63]
            p1 = spool.tile([BL, 1], f32, tag="p1")
            nc.vector._custom_dve(OPS["ANT_H3_TOP"], out=p1[:], in0=x0,
                                  in1=c8[:, 6:7], s0=c8[:, 3:4], s1=c8[:, 4:5],
                                  imm2=float(ch[8, 5]))
            t0p = spool.tile([BL, 1], f32, tag="t0p")
            nc.vector._custom_dve(OPS["ANT_H3_STEP"], out=t0p[:], in0=x0,
                                  in1=p1[:], s0=c8[:, 0:1], s1=c8[:, 1:2],
                                  imm2=float(ch[8, 2]))

            # ---------- gold: last term trans[STOP, tag_last] ----------
            idxT = spool.tile([BL, 1], i32, tag="idxT")
            nc.gpsimd.tensor_tensor(idxT[:], sTm1[:], leni_sb[:], op=OP.add)
            tgl = spool.tile([BL, 1], f32, tag="tgl")
            nc.gpsimd.indirect_dma_start(
                out=tgl[:], out_offset=None,
                in_=bass.AP(tagf, 0, [[1, BL * T], [1, 1]]),
                in_offset=bass.IndirectOffsetOnAxis(ap=idxT[:, 0:1], axis=0))
            f1 = spool.tile([BL, 1], f32, tag="f1")
            nc.vector.memset(f1[:], 1.0)
            xl = spool.tile([BL, 1], f32, tag="xl")
            nc.vector.tensor_tensor(xl[:], tgl[:], f1[:], op=OP.add)
            c7r = coefb[:, 49:56]
            q1 = spool.tile([BL, 1], f32, tag="q1")
            nc.vector._custom_dve(OPS["ANT_H3_TOP"], out=q1[:], in0=xl[:],
                                  in1=c7r[:, 6:7], s0=c7r[:, 3:4],
                                  s1=c7r[:, 4:5], imm2=float(ch[7, 5]))
            lastp = spool.tile([BL, 1], f32, tag="lastp")
            nc.vector._custom_dve(OPS["ANT_H3_STEP"], out=lastp[:], in0=xl[:],
                                  in1=q1[:], s0=c7r[:, 0:1], s1=c7r[:, 1:2],
                                  imm2=float(ch[7, 2]))

            # ---------- gathers ----------
            dg = spool.tile([BL, 1], bf16, tag="dg")
            nc.gpsimd.indirect_dma_start(
                out=dg[:], out_offset=None,
                in_=bass.AP(hist_d, 0, [[1, (C + 1) * 16 * COLS], [1, 1]]),
                in_offset=bass.IndirectOffsetOnAxis(ap=idxA[:, 0:1], axis=0))
            offg = spool.tile([BL, 1], f32, tag="offg")
            nc.gpsimd.indirect_dma_start(
                out=offg[:], out_offset=None,
                in_=bass.AP(offs_d, 0, [[1, 16 * COLS], [1, 1]]),
                in_offset=bass.IndirectOffsetOnAxis(ap=idxB[:, 0:1], axis=0))

            # ---------- finalize ----------
            lnv = spool.tile([BL, 1], f32, tag="lnv")
            nc.scalar.activation(lnv[:], dg[:], AF.Ln)
            fwd1 = spool.tile([BL, 1], f32, tag="fwd1")
            nc.vector.tensor_tensor(fwd1[:], lnv[:], offg[:], op=OP.add)
            fwd2 = spool.tile([BL, 1], f32, tag="fwd2")
            nc.vector.scalar_tensor_tensor(fwd2[:], lenf_sb[:], G, fwd1[:],
                                           op0=OP.mult, op1=OP.add)
            g2 = spool.tile([BL, 1], f32, tag="g2")
            nc.vector.tensor_tensor(g2[:], t0p[:], lastp[:], op=OP.add)
            g3 = spool.tile([BL, 1], f32, tag="g3")
            nc.vector.tensor_tensor(g3[:], acc[:], g2[:], op=OP.add)
            g4 = spool.tile([BL, 1], f32, tag="g4")
            nc.vector.tensor_tensor(g4[:], g3[:], facc[:], op=OP.add)
            res = spool.tile([BL, 1], f32, tag="res")
            nc.vector.tensor_tensor(res[:], fwd2[:], g4[:], op=OP.subtract)
            nc.sync.dma_start(outv[:, :], res[:])

    nc.finalize()
    return nc


def _coefs(transitions):
    tr = np.asarray(transitions, np.float64)
    V = np.vander(np.arange(1, 8, dtype=np.float64), 7, increasing=True)
    rows = [np.linalg.solve(V, tr[j, 0:7]) for j in range(7)]
    rows.append(np.linalg.solve(V, tr[STOP, 0:7]))
    rows.append(np.linalg.solve(V, tr[0:7, START]))
    return np.stack(rows).astype(np.float32)


def kernel(feats, transitions, tags, lengths):
    feats = np.ascontiguousarray(np.asarray(feats, dtype=np.float32))
    transitions = np.ascontiguousarray(np.asarray(transitions, dtype=np.float32))
    tags_f = np.ascontiguousarray(np.asarray(tags).astype(np.float32))
    len_f = np.ascontiguousarray(np.asarray(lengths).astype(np.float32).reshape(B, 1))
    len_i = np.ascontiguousarray(np.asarray(lengths).astype(np.int32).reshape(B, 1))
    coefs = np.ascontiguousarray(_coefs(transitions))

    key = ("nc", transitions.tobytes())
    if key not in _CACHE:
        _CACHE[key] = _build_bass(coefs.astype(np.float64))
    nc = _CACHE[key]

    from concourse.bass_utils import run_bass_kernel_spmd

    in_maps = []
    for c in range(NCORES):
        sl = slice(c * BL, (c + 1) * BL)
        in_maps.append({
            "feats": feats[sl],
            "tagf": tags_f[sl],
            "lenf": len_f[sl],
            "leni": len_i[sl],
            "trans": transitions,
            "coefs": coefs,
        })
    r = run_bass_kernel_spmd(nc, in_maps, core_ids=list(range(NCORES)),
                             trace=TRACE)
    if TRACE:
        _CACHE["last_result"] = r
    per_seq = np.concatenate([m["outv"].reshape(BL) for m in r.results])
    return np.float32(per_seq.mean(dtype=np.float64))
